# revision 78
# baseline (speedup 1.0000x reference)
"""Distributed causal multi-head attention for Trainium2 (8 NeuronCores).

Problem: x[2,2048,1024] @ w_qkv[1024,3072] -> 16-head causal attention
         -> @ w_out[1024,1024]. fp32 reference; device compute in bf16
         (fp32 PSUM accumulation).

Sharding (8 cores): core c owns heads {2c, 2c+1} for BOTH batches
(feature slice 128c..128c+128 of the qkv projections). Output rows are
sharded batch-major: core c owns rows [256c, 256c+256) of each batch.

Phase 2 runs one fused pass per BATCH: the two heads' S matmuls are
K=64 and live on disjoint PE row groups (head A at partitions 0:64 ->
tile rows 0:64, head B at 64:128 -> rows 64:128, tile_position derived
from base partitions), so emitting them back-to-back lets the PE
execute them concurrently (~2x on the score matmuls). The AllToAll is
split per batch ([8 slots, 128 dims, 256 rows] each): A2A#1 (batch 0)
overlaps the batch-1 pass; only A2A#2 is exposed at the tail.

Scheduling notes (the Tile scheduler reorders by readiness, and
semaphore wait thresholds are pinned at the simulated positions):
  - barriers are tiny AllToAlls, not AllGathers — mixing collective
    kinds desynchronized the Collectives-semaphore thresholds on hw
  - a sacrificial barrier absorbs the ~11us CC-firmware first-op ramp;
    no pre-tail barrier (it CC-serializes ahead of A2A#2, putting its
    own skew-inflated duration on the critical path)
  - normalize's partition broadcast uses a DRAM-bounce stride-0 DMA
    (GpSimd blocks behind in-flight collective_compute triggers)
  - the PV accumulators are copied out of PSUM before normalization so
    the banks recycle without waiting on the recip/broadcast chain
  - phase 1 runs only chunks 0-1 before the batch-0 pass (DMA-feed
    paced); chunks 2-3 and all of batch 1 are consume()-paced PE
    filler inside the passes, force-drained per chunk just in time

Device pipeline per core:
  P1: qT,kT = (w_qk stationary) @ xT chunks   [bf16, N=512 moving]
      vT    = (w_v stationary)  @ xT chunks -> PE-transpose -> V seq-major
      vaug  = [ones | pad | V_h] per j-tile   [ones row 0 => denominators]
  P2 (per batch b, i-chunk of 512, j-tile of 128): depth-2 software
      pipeline: S^T[j,i] for BOTH heads (two concurrent K=64 matmuls
      into one [128,2,512] PSUM tile) -> one ACT exp (scale fused,
      bf16) -> 128-col diagonal-block mask mul (DVE) -> per-head PV
      accumulate (row 0 = denominators) -> normalize -> DMA into the
      batch A2A buffer (slot split via pure-permutation APs).
  P3: per batch: AllToAll [8,128,256] bf16; gather to attr_sb; out
      rows = sum over 8 source K-tiles (full w_out contraction in one
      round, no partial staging); bf16 writeback.
"""
import os
import numpy as np
import ml_dtypes

import concourse.bass as bass
import concourse.bacc as bacc
import concourse.mybir as mybir
import concourse.tile as tile
from concourse.bass_utils import run_bass_kernel_spmd

F32 = mybir.dt.float32
BF16 = mybir.dt.bfloat16
AF = mybir.ActivationFunctionType

NC = 8           # cores
NB = 2           # batches
N = 2048         # seq len
D = 1024         # model dim
HPC = 2          # heads per core
HD = 64          # head dim
FS = HPC * HD    # per-core feature slice (128)
NFLAT = NB * N   # 4096 flattened rows
RPS = N // NC    # 256 rows per A2A slot (per batch)
SCALE = HD ** -0.5

_CACHED_NC = None


def build_graph():
    nc = bacc.Bacc("TRN2", target_bir_lowering=False, debug=False,
                   num_devices=NC)

    xT = nc.dram_tensor("xT", [128, NB, 8, N], BF16, kind="ExternalInput")
    wqkv = nc.dram_tensor("wqkv", [128, 8, 3 * FS], BF16, kind="ExternalInput")
    wout = nc.dram_tensor("wout", [128, 8, D], BF16, kind="ExternalInput")
    maskblk = nc.dram_tensor("maskblk", [128, 2, 128], BF16,
                             kind="ExternalInput")
    ident = nc.dram_tensor("ident", [128, 128], BF16, kind="ExternalInput")
    out = nc.dram_tensor("out", [NB, RPS, D], BF16, kind="ExternalOutput")

    with tile.TileContext(nc) as tc:
        _emit(nc, tc, xT, wqkv, wout, maskblk, ident, out)
    nc.compile()
    return nc


def _emit(nc, tc, xT, wqkv, wout, maskblk, ident, out):
    ctx_pools = []

    def pool(name, **kw):
        cm = tc.tile_pool(name=name, **kw)
        p = cm.__enter__()
        ctx_pools.append(cm)
        return p

    wpool = pool("weights", bufs=1)
    ptpool = pool("pt", bufs=8)
    spool = pool("stage", bufs=1)
    dpool = pool("dram", bufs=1, space="DRAM")
    pinit_cm = tc.tile_pool(name="psum_init", bufs=1, space="PSUM")
    pinit = pinit_cm.__enter__()

    # ---- persistent SBUF buffers ----
    xt_sb = wpool.tile([128, NB, 8, N], BF16)
    wqkv_sb = wpool.tile([128, 8, 3 * FS], BF16)
    wout_sb = wpool.tile([128, 8, D], BF16)
    maskblk_sb = wpool.tile([128, 2, 128], BF16)
    ident_sb = wpool.tile([128, 128], BF16)
    qkT_sb = wpool.tile([128, 2, NFLAT], BF16)          # [dims, q/k, b*N+i]
    # per j-tile [ones | junk | V_h]: row0=ones, rows 64:128 = V dims
    vaug_sb = wpool.tile([128, 32, HPC, 128], BF16)
    attr_sb = {b: wpool.tile([128, NC, RPS], BF16, name=f"attr{b}")
               for b in range(NB)}

    a2a_in = {b: dpool.tile([NC, FS, RPS], BF16, name=f"a2ai{b}")
              for b in range(NB)}
    a2a_out = {b: dpool.tile([NC, FS, RPS], BF16, name=f"a2ao{b}")
               for b in range(NB)}
    # barriers are implemented as tiny AllToAlls, NOT AllGathers: on this
    # runtime only AllToAll completions advance the Collectives semaphore,
    # so mixing kinds desynchronizes the tile framework's cumulative wait
    # thresholds (observed: the A2A#1 gather waiting on A2A#2).
    bar_in = dpool.tile([NC, 16], F32, name="bar_in")
    bar_out = dpool.tile([NC, 16], F32, name="bar_out")
    bar_in2 = dpool.tile([NC, 16], F32, name="bar_in2")
    bar_out2 = dpool.tile([NC, 16], F32, name="bar_out2")

    # startup DMAs: few LARGE transfers (big per-partition contiguous
    # descriptors — 4KB descriptors only reach ~half DMA throughput).
    nc.sync.dma_start(wqkv_sb[:, 0:2, :], wqkv[:, 0:2, :])
    nc.sync.dma_start(xt_sb[:, 0, 0:2, :], xT[:, 0, 0:2, :])
    nc.sync.dma_start(wqkv_sb[:, 2:8, :], wqkv[:, 2:8, :])
    nc.sync.dma_start(ident_sb[:], ident[:])
    for q in range(1, 4):
        nc.sync.dma_start(xt_sb[:, 0, 2 * q:2 * q + 2, :],
                          xT[:, 0, 2 * q:2 * q + 2, :])
    nc.sync.dma_start(maskblk_sb[:], maskblk[:])
    nc.vector.memset(vaug_sb[:, :, :, 0:1], 1.0)
    nc.vector.memset(vaug_sb[:, :, :, 1:64], 0.0)
    # explicit zero bias for Exp: avoids the shared const-0.0 SBUF tensor,
    # whose region aliases later pool tiles and trips false DMA/ACT races
    zbias = wpool.tile([128, 1], F32, name="zbias")
    nc.vector.memset(zbias[:], 0.0)
    bar_sb = wpool.tile([NC, 16], F32, name="bar_sb")
    nc.vector.memset(bar_sb[:], 0.0)
    nc.sync.dma_start(bar_in[:], bar_sb[:])
    # sacrificial barrier, ready immediately: the first collective after
    # the NEFF init barrier pays an ~11us CC-firmware ramp — let this one
    # absorb it during phase 1 instead of A2A#1.
    nc.gpsimd.collective_compute(
        "AllToAll", mybir.AluOpType.bypass,
        replica_groups=[list(range(NC))],
        ins=[bar_in.opt()], outs=[bar_out.opt()])

    def qk_mm(ps, b, ft, ic, dt):
        nc.tensor.matmul(
            ps[:],
            wqkv_sb[:, dt, 128 * ft:128 * (ft + 1)],
            xt_sb[:, b, dt, 512 * ic:512 * (ic + 1)],
            start=(dt == 0), stop=(dt == 7))

    def vt_mm(ps, b, ic, dt):
        nc.tensor.matmul(
            ps[:],
            wqkv_sb[:, dt, 2 * FS:3 * FS],
            xt_sb[:, b, dt, 512 * ic:512 * (ic + 1)],
            start=(dt == 0), stop=(dt == 7))

    def finish_qk(ps, b, ft, ic):
        nc.vector.tensor_copy(
            qkT_sb[:, ft, b * N + 512 * ic: b * N + 512 * (ic + 1)], ps[:])

    def finish_v(vps_list, b, psum_pool, ptag, pbufs):
        vT_bf = spool.tile([128, N], BF16, tag="vtb", bufs=2, name=f"vtb{b}")
        for ic in range(4):
            nc.vector.tensor_copy(vT_bf[:, 512 * ic:512 * (ic + 1)],
                                  vps_list[ic][:])
        for it in range(16):
            tp = psum_pool.tile([128, 128], BF16, tag=ptag, bufs=pbufs,
                                name=f"t_ps{b}_{it}")
            nc.tensor.transpose(tp[:], vT_bf[:, 128 * it:128 * (it + 1)],
                                ident_sb[:])
            nc.vector.tensor_copy(
                vaug_sb[:, 16 * b + it, :, 64:128],
                tp[:].rearrange("p (h c) -> p h c", h=HPC))

    # ---- warmup while the xT DMA streams in ----
    # preload the ACT exp table (first use costs ~1.3us)
    wsc = spool.tile([128, 1], BF16, tag="wsc", name="wsc")
    nc.scalar.activation(wsc[:], zbias[:], AF.Exp, bias=zbias[:], scale=1.0)

    # ---- Phase 1, batch 0, chunks 0-1 only: dt-outer passes, paced by
    # the xT DMA feed. Chunks 2-3 become pass-0 filler units so the
    # attention pass starts ~20us earlier. ----
    qk_ps = {(ft, ic): pinit.tile([128, 512], F32, tag="init",
                                  bufs=8, name=f"qk0_{ft}_{ic}")
             for ft in range(2) for ic in range(2)}
    v_ps0 = {ic: pinit.tile([128, 512], F32, tag="init", bufs=8,
                            name=f"v0_{ic}") for ic in range(2)}
    for dt in range(8):
        for ft in range(2):
            for ic in range(2):
                qk_mm(qk_ps[ft, ic], 0, ft, ic, dt)
        for ic in range(2):
            vt_mm(v_ps0[ic], 0, ic, dt)
    for ft in range(2):
        for ic in range(2):
            finish_qk(qk_ps[ft, ic], 0, ft, ic)
    vT_bf0 = spool.tile([128, N], BF16, tag="vtb", bufs=2, name="vtb0")
    for ic in range(2):
        nc.vector.tensor_copy(vT_bf0[:, 512 * ic:512 * (ic + 1)],
                              v_ps0[ic][:])
    for it in range(8):
        tp = pinit.tile([128, 128], BF16, tag="init", bufs=8,
                        name=f"t_ps0_{it}")
        nc.tensor.transpose(tp[:], vT_bf0[:, 128 * it:128 * (it + 1)],
                            ident_sb[:])
        nc.vector.tensor_copy(
            vaug_sb[:, it, :, 64:128],
            tp[:].rearrange("p (h c) -> p h c", h=HPC))
    pinit_cm.__exit__(None, None, None)
    ppool_cm = tc.tile_pool(name="psum", bufs=1, space="PSUM")
    ppool = ppool_cm.__enter__()

    # batch-1 x and the out-projection weights are gated behind batch-0's
    # first projection chunk (dummy WAW writes dependent on qkT) so their
    # DMA traffic doesn't compete with the batch-0 load that the phase-1
    # prefix is feed-limited by.
    nc.vector.tensor_copy(xt_sb[:, 1, 0, 0:1], qkT_sb[:, 0, 0:1])
    nc.sync.dma_start(xt_sb[:, 1, :, :], xT[:, 1, :, :])
    nc.vector.tensor_copy(wout_sb[:, 0, 0:1], qkT_sb[:, 0, 0:1])
    nc.sync.dma_start(wout_sb[:], wout[:])

    def p1_units(b, ic_from=0):
        """phase1_seq(b) from chunk ic_from on, decomposed into
        single-matmul emission units so it can be interleaved into a pass2
        as PE filler work. Ordered ic-major (31 units per ic) so a prefix
        makes i-chunk ic of batch b usable."""
        units = []
        state = {}

        def qk_group(ft, ic):
            def alloc():
                state[ft, ic] = ppool.tile([128, 512], F32, tag="mm", bufs=2,
                                           name=f"qk_ps{b}_{ft}_{ic}")
            for dt in range(8):
                def u(ft=ft, ic=ic, dt=dt):
                    if dt == 0:
                        alloc()
                    qk_mm(state[ft, ic], b, ft, ic, dt)
                units.append(u)
            units.append(lambda ft=ft, ic=ic: finish_qk(state[ft, ic], b, ft, ic))

        def v_group(ic):
            def alloc():
                state['v', ic] = ppool.tile([128, 512], F32, tag="mm", bufs=2,
                                            name=f"v_ps{b}_{ic}")
                if ic == ic_from:
                    state['vtb'] = spool.tile([128, N], BF16, tag="vtb",
                                              bufs=2, name=f"vtb{b}")
            for dt in range(8):
                def u(ic=ic, dt=dt):
                    if dt == 0:
                        alloc()
                    vt_mm(state['v', ic], b, ic, dt)
                units.append(u)

            def fin(ic=ic):
                nc.vector.tensor_copy(
                    state['vtb'][:, 512 * ic:512 * (ic + 1)],
                    state['v', ic][:])
            units.append(fin)

        def tr_unit(it):
            def tr(it=it):
                tp = ppool.tile([128, 128], BF16, tag="mm", bufs=2,
                                name=f"t_ps{b}_{it}")
                nc.tensor.transpose(tp[:], state['vtb'][:, 128 * it:128 * (it + 1)],
                                    ident_sb[:])
                nc.vector.tensor_copy(
                    vaug_sb[:, 16 * b + it, :, 64:128],
                    tp[:].rearrange("p (h c) -> p h c", h=HPC))
            units.append(tr)

        for ic in range(ic_from, 4):
            qk_group(0, ic)
            qk_group(1, ic)
            v_group(ic)
            for it in range(4 * ic, 4 * ic + 4):
                tr_unit(it)
        return units

    P1_UNITS_PER_IC = 31

    def proj_units(akey, n_rt, out_b, out_r0):
        """out-projection for 128*n_rt rows from attr_sb[akey]: full w_out
        contraction (8 source K-tiles) into [128,512] psum pairs, as
        filler units. Writes out[out_b, out_r0 + 128*rt ...]."""
        units = []
        state = {}
        for rt in range(n_rt):
            for u in range(8):
                def mm(rt=rt, u=u):
                    if u == 0:
                        for oc in range(2):
                            state[rt, oc] = ppool.tile(
                                [128, 512], F32, tag="mm", bufs=2,
                                name=f"op{akey}_{rt}_{oc}")
                    for oc in range(2):
                        nc.tensor.matmul(
                            state[rt, oc][:],
                            attr_sb[akey][:, u, 128 * rt:128 * (rt + 1)],
                            wout_sb[:, u, 512 * oc:512 * (oc + 1)],
                            start=(u == 0), stop=(u == 7))
                units.append(mm)

            def fin(rt=rt):
                ob = spool.tile([128, D], BF16, tag="ob", bufs=2,
                                name=f"ob{akey}_{rt}")
                for oc in range(2):
                    nc.vector.tensor_copy(ob[:, 512 * oc:512 * (oc + 1)],
                                          state[rt, oc][:])
                nc.sync.dma_start(
                    out[out_b, out_r0 + 128 * rt:out_r0 + 128 * (rt + 1), :],
                    ob[:])
            units.append(fin)
        return units

    last_anf = [None]
    cur_ptp = [None]

    def normalize(b, ic, pvs):
        # denom is pv row 0 (ones row of vaug), per head
        for h in range(HPC):
            if b == 1 and ic == 3:
                # last chunk: nothing follows, and the copy would sit on
                # the exposed A2A#2 trigger chain — normalize from PSUM
                pv = pvs[h, ic]
            else:
                # copy the accumulator out of PSUM first: releases the pv
                # bank immediately so the next chunk's first PV doesn't
                # stall behind the recip/broadcast/mul chain
                praw = spool.tile([128, 512], F32, tag="praw", bufs=4,
                                  name=f"pr{b}_{ic}_{h}")
                nc.vector.tensor_copy(praw[:], pvs[h, ic][:])
                pv = praw
            recip = spool.tile([1, 512], F32, tag="recip", bufs=2,
                               name=f"rc{b}_{ic}_{h}")
            nc.vector.reciprocal_approx_fast(recip[:], pv[0:1, :])
            if b == 1 and ic == 3:
                # GpSimd is free of in-flight collectives here: cheap
                # native partition broadcast (everywhere else the queue is
                # blocked behind a collective_compute trigger)
                bc = spool.tile([128, 512], F32, tag="bcg", bufs=2,
                                name=f"bcg{b}_{ic}_{h}")
                nc.gpsimd.partition_broadcast(bc[:], recip[:])
                bc_sl = bc[64:128, :]
            else:
                # during A2A#1 / the resync barrier the GpSimd queue is
                # blocked behind collective_compute — partition-broadcast
                # via a DRAM bounce + stride-0-source DMA instead (SBUF
                # APs reject zero partition stride; DRAM ones don't)
                rstage = dpool.tile([1, 512], F32, name=f"rst{b}_{ic}_{h}")
                nc.sync.dma_start(rstage[:], recip[:])
                bc = spool.tile([128, 512], F32, tag="bc", bufs=2,
                                name=f"bc{b}_{ic}_{h}")
                nc.sync.dma_start(bc[64:128, :],
                                  rstage[:].to_broadcast((64, 512)))
                bc_sl = bc[64:128, :]
            anf = spool.tile([128, 512], BF16, tag="an", bufs=4,
                             name=f"an{b}_{ic}_{h}")
            nc.vector.tensor_mul(anf[64:128, :], pv[64:128, :], bc_sl)
            last_anf[0] = anf
            # chunk ic covers slots 2ic (cols 0:256) and 2ic+1 (cols
            # 256:512); the slot split is a pure permutation on both sides.
            nc.sync.dma_start(
                a2a_in[b][2 * ic:2 * ic + 2, 64 * h:64 * (h + 1), :]
                .rearrange("s p r -> p s r"),
                anf[64:128, :].rearrange("p (s r) -> p s r", s=2))

    def pass_fused(b, consume=None, at_chunk=None, on_chunk_done=None):
        """causal attention for BOTH heads over all four 512-wide i-chunks
        of batch b, as one software-pipelined stream of j-tiles (the PV of
        tile k is emitted after the S of tile k+1, across chunk
        boundaries). Each j-tile's two heads' S matmuls are concurrent on
        the PE (disjoint row groups). `consume()` emits PE filler work
        once per j-tile; `at_chunk(ic)` runs before each chunk's first
        tile (for prerequisite draining)."""
        plan = [(ic, jt) for ic in range(4) for jt in range(4 * ic + 4)]
        pvs = {}

        def emit_pv(pend):
            pic, pjt, pcp, pptp = pend
            last = (pjt == 4 * pic + 3)
            for h in range(HPC):
                nc.tensor.matmul(pvs[h, pic][:, pcp:512],
                                 vaug_sb[:, 16 * b + pjt, h, :],
                                 pptp[:, h, pcp:512],
                                 start=(pjt == 0), stop=last)
            if last:
                normalize(b, pic, pvs)
                if on_chunk_done is not None:
                    on_chunk_done(pic)

        pend = []
        for ic, jt in plan:
            if jt == 0:
                if at_chunk is not None:
                    at_chunk(ic)
                for h in range(HPC):
                    pvs[h, ic] = ppool.tile([128, 512], F32, tag="pv",
                                            bufs=2, name=f"pv{b}_{ic}_{h}")
            q0 = jt - 4 * ic
            cp = 128 * q0 if q0 > 0 else 0
            sp = ppool.tile([128, 2, 512], F32, tag="s", bufs=2,
                            name=f"s{b}_{ic}_{jt}")
            ptp = ptpool.tile([128, 2, 512], BF16, tag="pt", bufs=16,
                              name=f"pt{b}_{ic}_{jt}")
            for h in range(HPC):
                nc.tensor.matmul(
                    sp[:, h, cp:512],
                    qkT_sb[64 * h:64 * (h + 1), 1,
                           b * N + 128 * jt: b * N + 128 * (jt + 1)],
                    qkT_sb[64 * h:64 * (h + 1), 0,
                           b * N + 512 * ic + cp: b * N + 512 * (ic + 1)],
                    start=True, stop=True)
            nc.scalar.activation(ptp[:, :, cp:512], sp[:, :, cp:512],
                                 AF.Exp, bias=zbias[:], scale=SCALE)
            if q0 >= 0:
                # diagonal tile: only the 128-col diagonal block needs the
                # causal mask (columns right of it are fully valid)
                nc.vector.tensor_mul(ptp[:, :, cp:cp + 128],
                                     ptp[:, :, cp:cp + 128], maskblk_sb[:])
            cur_ptp[0] = ptp
            if consume is not None:
                consume()
            # depth-2 software pipeline: the PV of tile k is emitted after
            # the S of tile k+2, so at chunk boundaries the next chunk's
            # first S/exp outrank the previous chunk's PV backlog in the
            # scheduler's program-order priority.
            if len(pend) >= 2:
                emit_pv(pend.pop(0))
            pend.append((ic, jt, cp, ptp))
        while pend:
            emit_pv(pend.pop(0))

    def do_a2a(key):
        nc.gpsimd.collective_compute(
            "AllToAll", mybir.AluOpType.bypass,
            replica_groups=[list(range(NC))],
            ins=[a2a_in[key].opt()], outs=[a2a_out[key].opt()])
        # gather [8,128,R] -> attr_sb[key] [128, 8, R]. Dispatched from
        # the GpSimd queue: it sits right after its own collective there
        # and fires the moment it completes (on the Sync queue these ended
        # up serialized behind LATER collectives' cumulative thresholds).
        eng = nc.scalar if key == 0 else nc.sync
        for half in range(2):
            eng.dma_start(
                attr_sb[key][:, 4 * half:4 * half + 4, :],
                a2a_out[key][4 * half:4 * half + 4]
                .rearrange("u p r -> p u r"))

    # ---- Phase 2, batch 0 ----
    # batch-0 chunks 2-3 QKV prep and then batch-1 QKV/V prep ride along
    # as PE filler; leftovers drain inside the batch-1 pass (force-drained
    # just in time per chunk).
    units0 = p1_units(0, ic_from=2)
    done0 = [0]
    units1 = p1_units(1)
    done1 = [0]

    def consume_p1(k):
        while k > 0:
            if done0[0] < len(units0):
                units0[done0[0]]()
                done0[0] += 1
            elif done1[0] < len(units1):
                units1[done1[0]]()
                done1[0] += 1
            else:
                break
            k -= 1

    def at_chunk_b0(ic):
        # chunk ic's S matmuls need q/k chunks <= ic and vaug tiles
        # <= 4*ic+3 of batch 0; units0 covers chunks 2-3 ic-major.
        need = P1_UNITS_PER_IC * max(0, ic - 1)
        consume_b0 = max(0, need - done0[0])
        while consume_b0 > 0 and done0[0] < len(units0):
            units0[done0[0]]()
            done0[0] += 1
            consume_b0 -= 1

    pass_fused(0, consume=lambda: consume_p1(3), at_chunk=at_chunk_b0)
    do_a2a(0)

    # ---- Phase 2, batch 1 ----
    # p1 leftovers fill the early tiles, then the batch-0 out-projection
    # rides along once A2A#1 + its gather have surely landed, then the
    # first-half batch-1 out-projection once A2A#2a has landed.
    unitsA = proj_units(0, 2, 0, 0)
    doneA = [0]
    jt_ctr = [0]
    A_GATE = 24

    # hold back the last row-tile group of the batch-0 out-projection: it
    # runs DURING the exposed A2A#2 window, keeping the PE p-state warm so
    # the batch-1 out-projection starts at full clock instead of ramping
    A_RESERVE = 9

    def consume_b1():
        jt_ctr[0] += 1
        if done1[0] < len(units1):
            consume_p1(2)
        elif jt_ctr[0] > A_GATE:
            for _ in range(2):
                if doneA[0] < len(unitsA) - A_RESERVE:
                    unitsA[doneA[0]]()
                    doneA[0] += 1

    def at_chunk_b1(ic):
        consume_p1(max(0, P1_UNITS_PER_IC * (ic + 1) - done1[0]))

    # no pre-tail barrier: it CC-serializes ahead of A2A#2, so when cores
    # ARE skewed its own (inflated) duration lands on the critical path —
    # letting A2A#2 absorb the skew directly costs no extra serialization
    pass_fused(1, consume=consume_b1, at_chunk=at_chunk_b1)

    # ---- Phase 3 tail: leftover batch-0 out-projection units, then the
    # A2A#2 (with the reserved units filling its window) + batch-1
    # out-projection + writeback ----
    while doneA[0] < len(unitsA) - A_RESERVE:
        unitsA[doneA[0]]()
        doneA[0] += 1
    do_a2a(1)
    while doneA[0] < len(unitsA):
        unitsA[doneA[0]]()
        doneA[0] += 1
    for u in proj_units(1, 2, 1, 0):
        u()

    for p in reversed(ctx_pools):
        p.__exit__(None, None, None)


def _host_inputs(x, w_qkv, w_out):
    x = np.asarray(x, dtype=np.float32)
    w_qkv = np.asarray(w_qkv, dtype=np.float32)
    w_out = np.asarray(w_out, dtype=np.float32)

    # xT[p, b, dt, i] = x[b, i, 128*dt + p]
    xTt = np.ascontiguousarray(
        x.transpose(2, 0, 1).reshape(8, 128, NB, N).transpose(1, 2, 0, 3)
    ).astype(ml_dtypes.bfloat16)

    wq, wk, wv = w_qkv[:, 0:D], w_qkv[:, D:2 * D], w_qkv[:, 2 * D:3 * D]

    # wout3[p, u, :] = w_out[128*u + p, :]
    wout3 = np.ascontiguousarray(
        w_out.reshape(8, 128, D).transpose(1, 0, 2)).astype(ml_dtypes.bfloat16)

    # diagonal-block causal mask, same for every diagonal j-tile:
    # keep iff (query col within block) >= (key partition)
    k_i = np.arange(128)[:, None]
    c_i = np.arange(128)[None, :]
    mblk = (c_i >= k_i)
    maskblk = np.ascontiguousarray(
        np.stack([mblk, mblk], axis=1)).astype(ml_dtypes.bfloat16)
    identity = np.eye(128, dtype=ml_dtypes.bfloat16)

    in_maps = []
    for c in range(NC):
        sl = slice(FS * c, FS * (c + 1))
        wq_c = np.concatenate([wq[:, sl], wk[:, sl], wv[:, sl]], axis=1)
        wq_c = np.ascontiguousarray(
            wq_c.astype(ml_dtypes.bfloat16).reshape(8, 128, 3 * FS)
            .transpose(1, 0, 2))
        in_maps.append({
            "xT": xTt,
            "wqkv": wq_c,
            "wout": wout3,
            "maskblk": maskblk,
            "ident": identity,
        })
    return in_maps


def run_hw(inputs, trace=False, **kw):
    """Run on 8 NeuronCores. Returns (full_output, BassKernelResults)."""
    global _CACHED_NC
    if _CACHED_NC is None:
        _CACHED_NC = build_graph()
    in_maps = _host_inputs(inputs["x"], inputs["w_qkv"], inputs["w_out"])
    res = run_bass_kernel_spmd(_CACHED_NC, in_maps,
                               core_ids=list(range(NC)), trace=trace, **kw)
    # core c's out is [NB, 256, D] = rows [256c, 256c+256) of each batch
    y = np.concatenate([np.asarray(res.results[c]["out"]) for c in range(NC)],
                       axis=1).astype(np.float32)
    return y, res


def kernel(**inputs):
    y, _ = run_hw(inputs, trace=bool(os.environ.get("BASS_TRACE")))
    return y


# revision 79
# speedup vs baseline: 1.0943x; 1.0943x over previous
"""Distributed causal multi-head attention for Trainium2 (8 NeuronCores).

Problem: x[2,2048,1024] @ w_qkv[1024,3072] -> 16-head causal attention
         -> @ w_out[1024,1024]. fp32 reference; device compute in bf16
         (fp32 PSUM accumulation).

Sharding (8 cores): core c owns heads {2c, 2c+1} for BOTH batches
(feature slice 128c..128c+128 of the qkv projections). Output rows are
sharded batch-major: core c owns rows [256c, 256c+256) of each batch.

Phase 2 runs one fused pass per BATCH: the two heads' S matmuls are
K=64 and live on disjoint PE row groups (head A at partitions 0:64 ->
tile rows 0:64, head B at 64:128 -> rows 64:128, tile_position derived
from base partitions), so emitting them back-to-back lets the PE
execute them concurrently (~2x on the score matmuls). The AllToAll is
split per batch ([8 slots, 128 dims, 256 rows] each): A2A#1 (batch 0)
overlaps the batch-1 pass; only A2A#2 is exposed at the tail.

Scheduling notes (the Tile scheduler reorders by readiness, and
semaphore wait thresholds are pinned at the simulated positions):
  - barriers are tiny AllToAlls, not AllGathers — mixing collective
    kinds desynchronized the Collectives-semaphore thresholds on hw
  - a sacrificial barrier absorbs the ~11us CC-firmware first-op ramp;
    no pre-tail barrier (it CC-serializes ahead of A2A#2, putting its
    own skew-inflated duration on the critical path)
  - normalize's partition broadcast uses a DRAM-bounce stride-0 DMA
    (GpSimd blocks behind in-flight collective_compute triggers)
  - the PV accumulators are copied out of PSUM before normalization so
    the banks recycle without waiting on the recip/broadcast chain
  - phase 1 runs only chunks 0-1 before the batch-0 pass (DMA-feed
    paced); chunks 2-3 and all of batch 1 are consume()-paced PE
    filler inside the passes, force-drained per chunk just in time

Device pipeline per core:
  P1: qT,kT = (w_qk stationary) @ xT chunks   [bf16, N=512 moving]
      vT    = (w_v stationary)  @ xT chunks -> PE-transpose -> V seq-major
      vaug  = [ones | pad | V_h] per j-tile   [ones row 0 => denominators]
  P2 (per batch b, i-chunk of 512, j-tile of 128): depth-2 software
      pipeline: S^T[j,i] for BOTH heads (two concurrent K=64 matmuls
      into one [128,2,512] PSUM tile) -> one ACT exp (scale fused,
      bf16) -> 128-col diagonal-block mask mul (DVE) -> per-head PV
      accumulate (row 0 = denominators) -> normalize -> DMA into the
      batch A2A buffer (slot split via pure-permutation APs).
  P3: per batch: AllToAll [8,128,256] bf16; gather to attr_sb; out
      rows = sum over 8 source K-tiles (full w_out contraction in one
      round, no partial staging); bf16 writeback.
"""
import os
import numpy as np
import ml_dtypes

import concourse.bass as bass
import concourse.bacc as bacc
import concourse.mybir as mybir
import concourse.tile as tile
from concourse.bass_utils import run_bass_kernel_spmd

F32 = mybir.dt.float32
BF16 = mybir.dt.bfloat16
AF = mybir.ActivationFunctionType

NC = 8           # cores
NB = 2           # batches
N = 2048         # seq len
D = 1024         # model dim
HPC = 2          # heads per core
HD = 64          # head dim
FS = HPC * HD    # per-core feature slice (128)
NFLAT = NB * N   # 4096 flattened rows
RPS = N // NC    # 256 rows per A2A slot (per batch)
SCALE = HD ** -0.5

_CACHED_NC = None


def build_graph():
    nc = bacc.Bacc("TRN2", target_bir_lowering=False, debug=False,
                   num_devices=NC)

    xT = nc.dram_tensor("xT", [128, NB, 8, N], BF16, kind="ExternalInput")
    wqkv = nc.dram_tensor("wqkv", [128, 8, 3 * FS], BF16, kind="ExternalInput")
    wout = nc.dram_tensor("wout", [128, 8, D], BF16, kind="ExternalInput")
    maskblk = nc.dram_tensor("maskblk", [128, 2, 128], BF16,
                             kind="ExternalInput")
    ident = nc.dram_tensor("ident", [128, 128], BF16, kind="ExternalInput")
    out = nc.dram_tensor("out", [NB, RPS, D], BF16, kind="ExternalOutput")

    with tile.TileContext(nc) as tc:
        _emit(nc, tc, xT, wqkv, wout, maskblk, ident, out)
    nc.compile()
    return nc


def _emit(nc, tc, xT, wqkv, wout, maskblk, ident, out):
    ctx_pools = []

    def pool(name, **kw):
        cm = tc.tile_pool(name=name, **kw)
        p = cm.__enter__()
        ctx_pools.append(cm)
        return p

    wpool = pool("weights", bufs=1)
    ptpool = pool("pt", bufs=8)
    spool = pool("stage", bufs=1)
    dpool = pool("dram", bufs=1, space="DRAM")
    pinit_cm = tc.tile_pool(name="psum_init", bufs=1, space="PSUM")
    pinit = pinit_cm.__enter__()

    # ---- persistent SBUF buffers ----
    xt_sb = wpool.tile([128, NB, 8, N], BF16)
    wqkv_sb = wpool.tile([128, 8, 3 * FS], BF16)
    wout_sb = wpool.tile([128, 8, D], BF16)
    maskblk_sb = wpool.tile([128, 2, 128], BF16)
    ident_sb = wpool.tile([128, 128], BF16)
    qkT_sb = wpool.tile([128, 2, NFLAT], BF16)          # [dims, q/k, b*N+i]
    # per j-tile [ones | junk | V_h]: row0=ones, rows 64:128 = V dims
    vaug_sb = wpool.tile([128, 32, HPC, 128], BF16)
    attr_sb = {b: wpool.tile([128, NC, RPS], BF16, name=f"attr{b}")
               for b in range(NB)}

    a2a_in = {b: dpool.tile([NC, FS, RPS], BF16, name=f"a2ai{b}")
              for b in range(NB)}
    a2a_out = {b: dpool.tile([NC, FS, RPS], BF16, name=f"a2ao{b}")
               for b in range(NB)}
    # barriers are implemented as tiny AllToAlls, NOT AllGathers: on this
    # runtime only AllToAll completions advance the Collectives semaphore,
    # so mixing kinds desynchronizes the tile framework's cumulative wait
    # thresholds (observed: the A2A#1 gather waiting on A2A#2).
    bar_in = dpool.tile([NC, 16], F32, name="bar_in")
    bar_out = dpool.tile([NC, 16], F32, name="bar_out")
    bar_in2 = dpool.tile([NC, 16], F32, name="bar_in2")
    bar_out2 = dpool.tile([NC, 16], F32, name="bar_out2")

    # startup DMAs: few LARGE transfers (big per-partition contiguous
    # descriptors — 4KB descriptors only reach ~half DMA throughput).
    nc.sync.dma_start(wqkv_sb[:, 0:2, :], wqkv[:, 0:2, :])
    nc.sync.dma_start(xt_sb[:, 0, 0:2, :], xT[:, 0, 0:2, :])
    nc.sync.dma_start(wqkv_sb[:, 2:8, :], wqkv[:, 2:8, :])
    nc.sync.dma_start(ident_sb[:], ident[:])
    for q in range(1, 4):
        nc.sync.dma_start(xt_sb[:, 0, 2 * q:2 * q + 2, :],
                          xT[:, 0, 2 * q:2 * q + 2, :])
    nc.sync.dma_start(maskblk_sb[:], maskblk[:])
    nc.vector.memset(vaug_sb[:, :, :, 0:1], 1.0)
    nc.vector.memset(vaug_sb[:, :, :, 1:64], 0.0)
    # explicit zero bias for Exp: avoids the shared const-0.0 SBUF tensor,
    # whose region aliases later pool tiles and trips false DMA/ACT races
    zbias = wpool.tile([128, 1], F32, name="zbias")
    nc.vector.memset(zbias[:], 0.0)
    bar_sb = wpool.tile([NC, 16], F32, name="bar_sb")
    nc.vector.memset(bar_sb[:], 0.0)
    nc.sync.dma_start(bar_in[:], bar_sb[:])
    # sacrificial barrier, ready immediately: the first collective after
    # the NEFF init barrier pays an ~11us CC-firmware ramp — let this one
    # absorb it during phase 1 instead of A2A#1.
    nc.gpsimd.collective_compute(
        "AllToAll", mybir.AluOpType.bypass,
        replica_groups=[list(range(NC))],
        ins=[bar_in.opt()], outs=[bar_out.opt()])

    def qk_mm(ps, b, ft, ic, dt):
        nc.tensor.matmul(
            ps[:],
            wqkv_sb[:, dt, 128 * ft:128 * (ft + 1)],
            xt_sb[:, b, dt, 512 * ic:512 * (ic + 1)],
            start=(dt == 0), stop=(dt == 7))

    def vt_mm(ps, b, ic, dt):
        nc.tensor.matmul(
            ps[:],
            wqkv_sb[:, dt, 2 * FS:3 * FS],
            xt_sb[:, b, dt, 512 * ic:512 * (ic + 1)],
            start=(dt == 0), stop=(dt == 7))

    def finish_qk(ps, b, ft, ic):
        nc.vector.tensor_copy(
            qkT_sb[:, ft, b * N + 512 * ic: b * N + 512 * (ic + 1)], ps[:])

    def finish_v(vps_list, b, psum_pool, ptag, pbufs):
        vT_bf = spool.tile([128, N], BF16, tag="vtb", bufs=2, name=f"vtb{b}")
        for ic in range(4):
            nc.vector.tensor_copy(vT_bf[:, 512 * ic:512 * (ic + 1)],
                                  vps_list[ic][:])
        for it in range(16):
            tp = psum_pool.tile([128, 128], BF16, tag=ptag, bufs=pbufs,
                                name=f"t_ps{b}_{it}")
            nc.tensor.transpose(tp[:], vT_bf[:, 128 * it:128 * (it + 1)],
                                ident_sb[:])
            nc.vector.tensor_copy(
                vaug_sb[:, 16 * b + it, :, 64:128],
                tp[:].rearrange("p (h c) -> p h c", h=HPC))

    # ---- warmup while the xT DMA streams in ----
    # preload the ACT exp table (first use costs ~1.3us)
    wsc = spool.tile([128, 1], BF16, tag="wsc", name="wsc")
    nc.scalar.activation(wsc[:], zbias[:], AF.Exp, bias=zbias[:], scale=1.0)

    # ---- Phase 1, batch 0, chunks 0-1 only: dt-outer passes, paced by
    # the xT DMA feed. Chunks 2-3 become pass-0 filler units so the
    # attention pass starts ~20us earlier. ----
    qk_ps = {(ft, ic): pinit.tile([128, 512], F32, tag="init",
                                  bufs=8, name=f"qk0_{ft}_{ic}")
             for ft in range(2) for ic in range(2)}
    v_ps0 = {ic: pinit.tile([128, 512], F32, tag="init", bufs=8,
                            name=f"v0_{ic}") for ic in range(2)}
    for dt in range(8):
        for ft in range(2):
            for ic in range(2):
                qk_mm(qk_ps[ft, ic], 0, ft, ic, dt)
        for ic in range(2):
            vt_mm(v_ps0[ic], 0, ic, dt)
    for ft in range(2):
        for ic in range(2):
            finish_qk(qk_ps[ft, ic], 0, ft, ic)
    vT_bf0 = spool.tile([128, N], BF16, tag="vtb", bufs=2, name="vtb0")
    for ic in range(2):
        nc.vector.tensor_copy(vT_bf0[:, 512 * ic:512 * (ic + 1)],
                              v_ps0[ic][:])
    for it in range(8):
        tp = pinit.tile([128, 128], BF16, tag="init", bufs=8,
                        name=f"t_ps0_{it}")
        nc.tensor.transpose(tp[:], vT_bf0[:, 128 * it:128 * (it + 1)],
                            ident_sb[:])
        nc.vector.tensor_copy(
            vaug_sb[:, it, :, 64:128],
            tp[:].rearrange("p (h c) -> p h c", h=HPC))
    pinit_cm.__exit__(None, None, None)
    ppool_cm = tc.tile_pool(name="psum", bufs=1, space="PSUM")
    ppool = ppool_cm.__enter__()

    # batch-1 x and the out-projection weights are gated behind batch-0's
    # first projection chunk (dummy WAW writes dependent on qkT) so their
    # DMA traffic doesn't compete with the batch-0 load that the phase-1
    # prefix is feed-limited by.
    nc.vector.tensor_copy(xt_sb[:, 1, 0, 0:1], qkT_sb[:, 0, 0:1])
    nc.sync.dma_start(xt_sb[:, 1, :, :], xT[:, 1, :, :])
    nc.vector.tensor_copy(wout_sb[:, 0, 0:1], qkT_sb[:, 0, 0:1])
    nc.sync.dma_start(wout_sb[:], wout[:])

    def p1_units(b, ic_from=0):
        """phase1_seq(b) from chunk ic_from on, decomposed into
        single-matmul emission units so it can be interleaved into a pass2
        as PE filler work. Ordered ic-major (31 units per ic) so a prefix
        makes i-chunk ic of batch b usable."""
        units = []
        state = {}

        def qk_group(ft, ic):
            def alloc():
                state[ft, ic] = ppool.tile([128, 512], F32, tag="mm", bufs=2,
                                           name=f"qk_ps{b}_{ft}_{ic}")
            for dt in range(8):
                def u(ft=ft, ic=ic, dt=dt):
                    if dt == 0:
                        alloc()
                    qk_mm(state[ft, ic], b, ft, ic, dt)
                units.append(u)
            units.append(lambda ft=ft, ic=ic: finish_qk(state[ft, ic], b, ft, ic))

        def v_group(ic):
            def alloc():
                state['v', ic] = ppool.tile([128, 512], F32, tag="mm", bufs=2,
                                            name=f"v_ps{b}_{ic}")
                if ic == ic_from:
                    state['vtb'] = spool.tile([128, N], BF16, tag="vtb",
                                              bufs=2, name=f"vtb{b}")
            for dt in range(8):
                def u(ic=ic, dt=dt):
                    if dt == 0:
                        alloc()
                    vt_mm(state['v', ic], b, ic, dt)
                units.append(u)

            def fin(ic=ic):
                nc.vector.tensor_copy(
                    state['vtb'][:, 512 * ic:512 * (ic + 1)],
                    state['v', ic][:])
            units.append(fin)

        def tr_unit(it):
            def tr(it=it):
                tp = ppool.tile([128, 128], BF16, tag="mm", bufs=2,
                                name=f"t_ps{b}_{it}")
                nc.tensor.transpose(tp[:], state['vtb'][:, 128 * it:128 * (it + 1)],
                                    ident_sb[:])
                nc.vector.tensor_copy(
                    vaug_sb[:, 16 * b + it, :, 64:128],
                    tp[:].rearrange("p (h c) -> p h c", h=HPC))
            units.append(tr)

        for ic in range(ic_from, 4):
            qk_group(0, ic)
            qk_group(1, ic)
            v_group(ic)
            for it in range(4 * ic, 4 * ic + 4):
                tr_unit(it)
        return units

    P1_UNITS_PER_IC = 31

    def proj_units(akey, n_rt, out_b, out_r0):
        """out-projection for 128*n_rt rows from attr_sb[akey]: full w_out
        contraction (8 source K-tiles) into [128,512] psum pairs, as
        filler units. Writes out[out_b, out_r0 + 128*rt ...]."""
        units = []
        state = {}
        for rt in range(n_rt):
            for u in range(8):
                def mm(rt=rt, u=u):
                    if u == 0:
                        for oc in range(2):
                            state[rt, oc] = ppool.tile(
                                [128, 512], F32, tag="mm", bufs=2,
                                name=f"op{akey}_{rt}_{oc}")
                    for oc in range(2):
                        nc.tensor.matmul(
                            state[rt, oc][:],
                            attr_sb[akey][:, u, 128 * rt:128 * (rt + 1)],
                            wout_sb[:, u, 512 * oc:512 * (oc + 1)],
                            start=(u == 0), stop=(u == 7))
                units.append(mm)

            def fin(rt=rt):
                ob = spool.tile([128, D], BF16, tag="ob", bufs=2,
                                name=f"ob{akey}_{rt}")
                for oc in range(2):
                    nc.vector.tensor_copy(ob[:, 512 * oc:512 * (oc + 1)],
                                          state[rt, oc][:])
                nc.sync.dma_start(
                    out[out_b, out_r0 + 128 * rt:out_r0 + 128 * (rt + 1), :],
                    ob[:])
            units.append(fin)
        return units

    last_anf = [None]
    cur_ptp = [None]

    def normalize(b, ic, pvs):
        # denom is pv row 0 (ones row of vaug), per head
        for h in range(HPC):
            if b == 1 and ic == 3:
                # last chunk: nothing follows, and the copy would sit on
                # the exposed A2A#2 trigger chain — normalize from PSUM
                pv = pvs[h, ic]
            else:
                # copy the accumulator out of PSUM first: releases the pv
                # bank immediately so the next chunk's first PV doesn't
                # stall behind the recip/broadcast/mul chain
                praw = spool.tile([128, 512], F32, tag="praw", bufs=4,
                                  name=f"pr{b}_{ic}_{h}")
                nc.vector.tensor_copy(praw[:], pvs[h, ic][:])
                pv = praw
            recip = spool.tile([1, 512], F32, tag="recip", bufs=2,
                               name=f"rc{b}_{ic}_{h}")
            nc.vector.reciprocal_approx_fast(recip[:], pv[0:1, :])
            if b == 1 and ic == 3:
                # GpSimd is free of in-flight collectives here: cheap
                # native partition broadcast (everywhere else the queue is
                # blocked behind a collective_compute trigger)
                bc = spool.tile([128, 512], F32, tag="bcg", bufs=2,
                                name=f"bcg{b}_{ic}_{h}")
                nc.gpsimd.partition_broadcast(bc[:], recip[:])
                bc_sl = bc[64:128, :]
            else:
                # during A2A#1 / the resync barrier the GpSimd queue is
                # blocked behind collective_compute — partition-broadcast
                # via a DRAM bounce + stride-0-source DMA instead (SBUF
                # APs reject zero partition stride; DRAM ones don't)
                rstage = dpool.tile([1, 512], F32, name=f"rst{b}_{ic}_{h}")
                nc.sync.dma_start(rstage[:], recip[:])
                bc = spool.tile([128, 512], F32, tag="bc", bufs=2,
                                name=f"bc{b}_{ic}_{h}")
                nc.sync.dma_start(bc[64:128, :],
                                  rstage[:].to_broadcast((64, 512)))
                bc_sl = bc[64:128, :]
            anf = spool.tile([128, 512], BF16, tag="an", bufs=4,
                             name=f"an{b}_{ic}_{h}")
            nc.vector.tensor_mul(anf[64:128, :], pv[64:128, :], bc_sl)
            last_anf[0] = anf
            # chunk ic covers slots 2ic (cols 0:256) and 2ic+1 (cols
            # 256:512); the slot split is a pure permutation on both sides.
            nc.sync.dma_start(
                a2a_in[b][2 * ic:2 * ic + 2, 64 * h:64 * (h + 1), :]
                .rearrange("s p r -> p s r"),
                anf[64:128, :].rearrange("p (s r) -> p s r", s=2))

    def pass_fused(b, consume=None, at_chunk=None, on_chunk_done=None):
        """causal attention for BOTH heads over all four 512-wide i-chunks
        of batch b, as one software-pipelined stream of j-tiles (the PV of
        tile k is emitted after the S of tile k+1, across chunk
        boundaries). Each j-tile's two heads' S matmuls are concurrent on
        the PE (disjoint row groups). `consume()` emits PE filler work
        once per j-tile; `at_chunk(ic)` runs before each chunk's first
        tile (for prerequisite draining)."""
        plan = [(ic, jt) for ic in range(4) for jt in range(4 * ic + 4)]
        pvs = {}

        def emit_pv(pend):
            pic, pjt, pcp, pptp = pend
            last = (pjt == 4 * pic + 3)
            for h in range(HPC):
                nc.tensor.matmul(pvs[h, pic][:, pcp:512],
                                 vaug_sb[:, 16 * b + pjt, h, :],
                                 pptp[:, h, pcp:512],
                                 start=(pjt == 0), stop=last)
            if last:
                normalize(b, pic, pvs)
                if on_chunk_done is not None:
                    on_chunk_done(pic)

        pend = []
        for ic, jt in plan:
            if jt == 0:
                if at_chunk is not None:
                    at_chunk(ic)
                for h in range(HPC):
                    pvs[h, ic] = ppool.tile([128, 512], F32, tag="pv",
                                            bufs=2, name=f"pv{b}_{ic}_{h}")
            q0 = jt - 4 * ic
            cp = 128 * q0 if q0 > 0 else 0
            sp = ppool.tile([128, 2, 512], F32, tag="s", bufs=2,
                            name=f"s{b}_{ic}_{jt}")
            ptp = ptpool.tile([128, 2, 512], BF16, tag="pt", bufs=16,
                              name=f"pt{b}_{ic}_{jt}")
            for h in range(HPC):
                nc.tensor.matmul(
                    sp[:, h, cp:512],
                    qkT_sb[64 * h:64 * (h + 1), 1,
                           b * N + 128 * jt: b * N + 128 * (jt + 1)],
                    qkT_sb[64 * h:64 * (h + 1), 0,
                           b * N + 512 * ic + cp: b * N + 512 * (ic + 1)],
                    start=True, stop=True)
            nc.scalar.activation(ptp[:, :, cp:512], sp[:, :, cp:512],
                                 AF.Exp, bias=zbias[:], scale=SCALE)
            if q0 >= 0:
                # diagonal tile: only the 128-col diagonal block needs the
                # causal mask (columns right of it are fully valid)
                nc.vector.tensor_mul(ptp[:, :, cp:cp + 128],
                                     ptp[:, :, cp:cp + 128], maskblk_sb[:])
            cur_ptp[0] = ptp
            if consume is not None:
                consume()
            # depth-2 software pipeline: the PV of tile k is emitted after
            # the S of tile k+2, so at chunk boundaries the next chunk's
            # first S/exp outrank the previous chunk's PV backlog in the
            # scheduler's program-order priority.
            if len(pend) >= 2:
                emit_pv(pend.pop(0))
            pend.append((ic, jt, cp, ptp))
        while pend:
            emit_pv(pend.pop(0))

    def do_a2a(key):
        nc.gpsimd.collective_compute(
            "AllToAll", mybir.AluOpType.bypass,
            replica_groups=[list(range(NC))],
            ins=[a2a_in[key].opt()], outs=[a2a_out[key].opt()])
        # gather [8,128,R] -> attr_sb[key] [128, 8, R]. Dispatched from
        # the GpSimd queue: it sits right after its own collective there
        # and fires the moment it completes (on the Sync queue these ended
        # up serialized behind LATER collectives' cumulative thresholds).
        eng = nc.scalar if key == 0 else nc.sync
        for half in range(2):
            eng.dma_start(
                attr_sb[key][:, 4 * half:4 * half + 4, :],
                a2a_out[key][4 * half:4 * half + 4]
                .rearrange("u p r -> p u r"))

    # ---- Phase 2, batch 0 ----
    # batch-0 chunks 2-3 QKV prep and then batch-1 QKV/V prep ride along
    # as PE filler; leftovers drain inside the batch-1 pass (force-drained
    # just in time per chunk).
    units0 = p1_units(0, ic_from=2)
    done0 = [0]
    units1 = p1_units(1)
    done1 = [0]

    def consume_p1(k):
        while k > 0:
            if done0[0] < len(units0):
                units0[done0[0]]()
                done0[0] += 1
            elif done1[0] < len(units1):
                units1[done1[0]]()
                done1[0] += 1
            else:
                break
            k -= 1

    def at_chunk_b0(ic):
        # chunk ic's S matmuls need q/k chunks <= ic and vaug tiles
        # <= 4*ic+3 of batch 0; units0 covers chunks 2-3 ic-major.
        need = P1_UNITS_PER_IC * max(0, ic - 1)
        consume_b0 = max(0, need - done0[0])
        while consume_b0 > 0 and done0[0] < len(units0):
            units0[done0[0]]()
            done0[0] += 1
            consume_b0 -= 1

    pass_fused(0, consume=lambda: consume_p1(3), at_chunk=at_chunk_b0)
    do_a2a(0)

    # ---- Phase 2, batch 1 ----
    # p1 leftovers fill the early tiles, then the batch-0 out-projection
    # rides along once A2A#1 + its gather have surely landed, then the
    # first-half batch-1 out-projection once A2A#2a has landed.
    unitsA = proj_units(0, 2, 0, 0)
    doneA = [0]
    jt_ctr = [0]
    A_GATE = 999

    # hold back the last row-tile group of the batch-0 out-projection: it
    # runs DURING the exposed A2A#2 window, keeping the PE p-state warm so
    # the batch-1 out-projection starts at full clock instead of ramping
    A_RESERVE = 9

    def consume_b1():
        jt_ctr[0] += 1
        if done1[0] < len(units1):
            consume_p1(2)
        elif jt_ctr[0] > A_GATE:
            for _ in range(2):
                if doneA[0] < len(unitsA) - A_RESERVE:
                    unitsA[doneA[0]]()
                    doneA[0] += 1

    def at_chunk_b1(ic):
        consume_p1(max(0, P1_UNITS_PER_IC * (ic + 1) - done1[0]))

    # no pre-tail barrier: it CC-serializes ahead of A2A#2, so when cores
    # ARE skewed its own (inflated) duration lands on the critical path —
    # letting A2A#2 absorb the skew directly costs no extra serialization
    pass_fused(1, consume=consume_b1, at_chunk=at_chunk_b1)

    # ---- Phase 3 tail: leftover batch-0 out-projection units, then the
    # A2A#2 (with the reserved units filling its window) + batch-1
    # out-projection + writeback ----
    while doneA[0] < len(unitsA) - A_RESERVE:
        unitsA[doneA[0]]()
        doneA[0] += 1
    do_a2a(1)
    while doneA[0] < len(unitsA):
        unitsA[doneA[0]]()
        doneA[0] += 1
    for u in proj_units(1, 2, 1, 0):
        u()

    for p in reversed(ctx_pools):
        p.__exit__(None, None, None)


def _host_inputs(x, w_qkv, w_out):
    x = np.asarray(x, dtype=np.float32)
    w_qkv = np.asarray(w_qkv, dtype=np.float32)
    w_out = np.asarray(w_out, dtype=np.float32)

    # xT[p, b, dt, i] = x[b, i, 128*dt + p]
    xTt = np.ascontiguousarray(
        x.transpose(2, 0, 1).reshape(8, 128, NB, N).transpose(1, 2, 0, 3)
    ).astype(ml_dtypes.bfloat16)

    wq, wk, wv = w_qkv[:, 0:D], w_qkv[:, D:2 * D], w_qkv[:, 2 * D:3 * D]

    # wout3[p, u, :] = w_out[128*u + p, :]
    wout3 = np.ascontiguousarray(
        w_out.reshape(8, 128, D).transpose(1, 0, 2)).astype(ml_dtypes.bfloat16)

    # diagonal-block causal mask, same for every diagonal j-tile:
    # keep iff (query col within block) >= (key partition)
    k_i = np.arange(128)[:, None]
    c_i = np.arange(128)[None, :]
    mblk = (c_i >= k_i)
    maskblk = np.ascontiguousarray(
        np.stack([mblk, mblk], axis=1)).astype(ml_dtypes.bfloat16)
    identity = np.eye(128, dtype=ml_dtypes.bfloat16)

    in_maps = []
    for c in range(NC):
        sl = slice(FS * c, FS * (c + 1))
        wq_c = np.concatenate([wq[:, sl], wk[:, sl], wv[:, sl]], axis=1)
        wq_c = np.ascontiguousarray(
            wq_c.astype(ml_dtypes.bfloat16).reshape(8, 128, 3 * FS)
            .transpose(1, 0, 2))
        in_maps.append({
            "xT": xTt,
            "wqkv": wq_c,
            "wout": wout3,
            "maskblk": maskblk,
            "ident": identity,
        })
    return in_maps


def run_hw(inputs, trace=False, **kw):
    """Run on 8 NeuronCores. Returns (full_output, BassKernelResults)."""
    global _CACHED_NC
    if _CACHED_NC is None:
        _CACHED_NC = build_graph()
    in_maps = _host_inputs(inputs["x"], inputs["w_qkv"], inputs["w_out"])
    res = run_bass_kernel_spmd(_CACHED_NC, in_maps,
                               core_ids=list(range(NC)), trace=trace, **kw)
    # core c's out is [NB, 256, D] = rows [256c, 256c+256) of each batch
    y = np.concatenate([np.asarray(res.results[c]["out"]) for c in range(NC)],
                       axis=1).astype(np.float32)
    return y, res


def kernel(**inputs):
    y, _ = run_hw(inputs, trace=bool(os.environ.get("BASS_TRACE")))
    return y


# revision 82
# speedup vs baseline: 1.0981x; 1.0035x over previous
"""Distributed causal multi-head attention for Trainium2 (8 NeuronCores).

Problem: x[2,2048,1024] @ w_qkv[1024,3072] -> 16-head causal attention
         -> @ w_out[1024,1024]. fp32 reference; device compute in bf16
         (fp32 PSUM accumulation).

Sharding (8 cores): core c owns heads {2c, 2c+1} for BOTH batches
(feature slice 128c..128c+128 of the qkv projections). Output rows are
sharded batch-major: core c owns rows [256c, 256c+256) of each batch.

Phase 2 runs one fused pass per BATCH: the two heads' S matmuls are
K=64 and live on disjoint PE row groups (head A at partitions 0:64 ->
tile rows 0:64, head B at 64:128 -> rows 64:128, tile_position derived
from base partitions), so emitting them back-to-back lets the PE
execute them concurrently (~2x on the score matmuls). The AllToAll is
split per batch ([8 slots, 128 dims, 256 rows] each): A2A#1 (batch 0)
overlaps the batch-1 pass; only A2A#2 is exposed at the tail.

Scheduling notes (the Tile scheduler reorders by readiness, and
semaphore wait thresholds are pinned at the simulated positions):
  - barriers are tiny AllToAlls, not AllGathers — mixing collective
    kinds desynchronized the Collectives-semaphore thresholds on hw
  - a sacrificial barrier absorbs the ~11us CC-firmware first-op ramp;
    no pre-tail barrier (it CC-serializes ahead of A2A#2, putting its
    own skew-inflated duration on the critical path)
  - normalize's partition broadcast uses a DRAM-bounce stride-0 DMA
    (GpSimd blocks behind in-flight collective_compute triggers)
  - the PV accumulators are copied out of PSUM before normalization so
    the banks recycle without waiting on the recip/broadcast chain
  - phase 1 runs only chunks 0-1 before the batch-0 pass (DMA-feed
    paced); chunks 2-3 and all of batch 1 are consume()-paced PE
    filler inside the passes, force-drained per chunk just in time

Device pipeline per core:
  P1: qT,kT = (w_qk stationary) @ xT chunks   [bf16, N=512 moving]
      vT    = (w_v stationary)  @ xT chunks -> PE-transpose -> V seq-major
      vaug  = [ones | pad | V_h] per j-tile   [ones row 0 => denominators]
  P2 (per batch b, i-chunk of 512, j-tile of 128): depth-2 software
      pipeline: S^T[j,i] for BOTH heads (two concurrent K=64 matmuls
      into one [128,2,512] PSUM tile) -> one ACT exp (scale fused,
      bf16) -> 128-col diagonal-block mask mul (DVE) -> per-head PV
      accumulate (row 0 = denominators) -> normalize -> DMA into the
      batch A2A buffer (slot split via pure-permutation APs).
  P3: per batch: AllToAll [8,128,256] bf16; gather to attr_sb; out
      rows = sum over 8 source K-tiles (full w_out contraction in one
      round, no partial staging); bf16 writeback.
"""
import os
import numpy as np
import ml_dtypes

import concourse.bass as bass
import concourse.bacc as bacc
import concourse.mybir as mybir
import concourse.tile as tile
from concourse.bass_utils import run_bass_kernel_spmd

F32 = mybir.dt.float32
BF16 = mybir.dt.bfloat16
AF = mybir.ActivationFunctionType

NC = 8           # cores
NB = 2           # batches
N = 2048         # seq len
D = 1024         # model dim
HPC = 2          # heads per core
HD = 64          # head dim
FS = HPC * HD    # per-core feature slice (128)
NFLAT = NB * N   # 4096 flattened rows
RPS = N // NC    # 256 rows per A2A slot (per batch)
SCALE = HD ** -0.5

_CACHED_NC = None


def build_graph():
    nc = bacc.Bacc("TRN2", target_bir_lowering=False, debug=False,
                   num_devices=NC)

    xT = nc.dram_tensor("xT", [128, NB, 8, N], BF16, kind="ExternalInput")
    wqkv = nc.dram_tensor("wqkv", [128, 8, 3 * FS], BF16, kind="ExternalInput")
    wout = nc.dram_tensor("wout", [128, 8, D], BF16, kind="ExternalInput")
    maskblk = nc.dram_tensor("maskblk", [128, 2, 128], BF16,
                             kind="ExternalInput")
    ident = nc.dram_tensor("ident", [128, 128], BF16, kind="ExternalInput")
    out = nc.dram_tensor("out", [NB, RPS, D], BF16, kind="ExternalOutput")

    with tile.TileContext(nc) as tc:
        _emit(nc, tc, xT, wqkv, wout, maskblk, ident, out)
    nc.compile()
    return nc


def _emit(nc, tc, xT, wqkv, wout, maskblk, ident, out):
    ctx_pools = []

    def pool(name, **kw):
        cm = tc.tile_pool(name=name, **kw)
        p = cm.__enter__()
        ctx_pools.append(cm)
        return p

    wpool = pool("weights", bufs=1)
    ptpool = pool("pt", bufs=8)
    spool = pool("stage", bufs=1)
    dpool = pool("dram", bufs=1, space="DRAM")
    pinit_cm = tc.tile_pool(name="psum_init", bufs=1, space="PSUM")
    pinit = pinit_cm.__enter__()

    # ---- persistent SBUF buffers ----
    xt_sb = wpool.tile([128, NB, 8, N], BF16)
    wqkv_sb = wpool.tile([128, 8, 3 * FS], BF16)
    wout_sb = wpool.tile([128, 8, D], BF16)
    maskblk_sb = wpool.tile([128, 2, 128], BF16)
    ident_sb = wpool.tile([128, 128], BF16)
    qkT_sb = wpool.tile([128, 2, NFLAT], BF16)          # [dims, q/k, b*N+i]
    # per j-tile [ones | junk | V_h]: row0=ones, rows 64:128 = V dims
    vaug_sb = wpool.tile([128, 32, HPC, 128], BF16)
    attr_sb = {b: wpool.tile([128, NC, RPS], BF16, name=f"attr{b}")
               for b in range(NB)}

    a2a_in = {b: dpool.tile([NC, FS, RPS], BF16, name=f"a2ai{b}")
              for b in range(NB)}
    a2a_out = {b: dpool.tile([NC, FS, RPS], BF16, name=f"a2ao{b}")
               for b in range(NB)}
    # barriers are implemented as tiny AllToAlls, NOT AllGathers: on this
    # runtime only AllToAll completions advance the Collectives semaphore,
    # so mixing kinds desynchronizes the tile framework's cumulative wait
    # thresholds (observed: the A2A#1 gather waiting on A2A#2).
    bar_in = dpool.tile([NC, 16], F32, name="bar_in")
    bar_out = dpool.tile([NC, 16], F32, name="bar_out")
    bar_in2 = dpool.tile([NC, 16], F32, name="bar_in2")
    bar_out2 = dpool.tile([NC, 16], F32, name="bar_out2")

    # startup DMAs: few LARGE transfers (big per-partition contiguous
    # descriptors — 4KB descriptors only reach ~half DMA throughput).
    nc.sync.dma_start(wqkv_sb[:, 0:2, :], wqkv[:, 0:2, :])
    nc.sync.dma_start(xt_sb[:, 0, 0:2, :], xT[:, 0, 0:2, :])
    nc.sync.dma_start(wqkv_sb[:, 2:8, :], wqkv[:, 2:8, :])
    nc.sync.dma_start(ident_sb[:], ident[:])
    for q in range(1, 4):
        nc.sync.dma_start(xt_sb[:, 0, 2 * q:2 * q + 2, :],
                          xT[:, 0, 2 * q:2 * q + 2, :])
    nc.sync.dma_start(maskblk_sb[:], maskblk[:])
    nc.vector.memset(vaug_sb[:, :, :, 0:1], 1.0)
    nc.vector.memset(vaug_sb[:, :, :, 1:64], 0.0)
    # explicit zero bias for Exp: avoids the shared const-0.0 SBUF tensor,
    # whose region aliases later pool tiles and trips false DMA/ACT races
    zbias = wpool.tile([128, 1], F32, name="zbias")
    nc.vector.memset(zbias[:], 0.0)
    bar_sb = wpool.tile([NC, 16], F32, name="bar_sb")
    nc.vector.memset(bar_sb[:], 0.0)
    nc.sync.dma_start(bar_in[:], bar_sb[:])
    # sacrificial barrier, ready immediately: the first collective after
    # the NEFF init barrier pays an ~11us CC-firmware ramp — let this one
    # absorb it during phase 1 instead of A2A#1.
    nc.gpsimd.collective_compute(
        "AllToAll", mybir.AluOpType.bypass,
        replica_groups=[list(range(NC))],
        ins=[bar_in.opt()], outs=[bar_out.opt()])

    def qk_mm(ps, b, ft, ic, dt):
        nc.tensor.matmul(
            ps[:],
            wqkv_sb[:, dt, 128 * ft:128 * (ft + 1)],
            xt_sb[:, b, dt, 512 * ic:512 * (ic + 1)],
            start=(dt == 0), stop=(dt == 7))

    def vt_mm(ps, b, ic, dt):
        nc.tensor.matmul(
            ps[:],
            wqkv_sb[:, dt, 2 * FS:3 * FS],
            xt_sb[:, b, dt, 512 * ic:512 * (ic + 1)],
            start=(dt == 0), stop=(dt == 7))

    def finish_qk(ps, b, ft, ic):
        nc.vector.tensor_copy(
            qkT_sb[:, ft, b * N + 512 * ic: b * N + 512 * (ic + 1)], ps[:])

    def finish_v(vps_list, b, psum_pool, ptag, pbufs):
        vT_bf = spool.tile([128, N], BF16, tag="vtb", bufs=2, name=f"vtb{b}")
        for ic in range(4):
            nc.vector.tensor_copy(vT_bf[:, 512 * ic:512 * (ic + 1)],
                                  vps_list[ic][:])
        for it in range(16):
            tp = psum_pool.tile([128, 128], BF16, tag=ptag, bufs=pbufs,
                                name=f"t_ps{b}_{it}")
            nc.tensor.transpose(tp[:], vT_bf[:, 128 * it:128 * (it + 1)],
                                ident_sb[:])
            nc.vector.tensor_copy(
                vaug_sb[:, 16 * b + it, :, 64:128],
                tp[:].rearrange("p (h c) -> p h c", h=HPC))

    # ---- warmup while the xT DMA streams in ----
    # preload the ACT exp table (first use costs ~1.3us)
    wsc = spool.tile([128, 1], BF16, tag="wsc", name="wsc")
    nc.scalar.activation(wsc[:], zbias[:], AF.Exp, bias=zbias[:], scale=1.0)

    # ---- Phase 1, batch 0, chunks 0-1 only: dt-outer passes, paced by
    # the xT DMA feed. Chunks 2-3 become pass-0 filler units so the
    # attention pass starts ~20us earlier. ----
    qk_ps = {(ft, ic): pinit.tile([128, 512], F32, tag="init",
                                  bufs=8, name=f"qk0_{ft}_{ic}")
             for ft in range(2) for ic in range(2)}
    v_ps0 = {ic: pinit.tile([128, 512], F32, tag="init", bufs=8,
                            name=f"v0_{ic}") for ic in range(2)}
    for dt in range(8):
        for ft in range(2):
            for ic in range(2):
                qk_mm(qk_ps[ft, ic], 0, ft, ic, dt)
        for ic in range(2):
            vt_mm(v_ps0[ic], 0, ic, dt)
    for ft in range(2):
        for ic in range(2):
            finish_qk(qk_ps[ft, ic], 0, ft, ic)
    vT_bf0 = spool.tile([128, N], BF16, tag="vtb", bufs=2, name="vtb0")
    for ic in range(2):
        nc.vector.tensor_copy(vT_bf0[:, 512 * ic:512 * (ic + 1)],
                              v_ps0[ic][:])
    for it in range(8):
        tp = pinit.tile([128, 128], BF16, tag="init", bufs=8,
                        name=f"t_ps0_{it}")
        nc.tensor.transpose(tp[:], vT_bf0[:, 128 * it:128 * (it + 1)],
                            ident_sb[:])
        nc.vector.tensor_copy(
            vaug_sb[:, it, :, 64:128],
            tp[:].rearrange("p (h c) -> p h c", h=HPC))
    pinit_cm.__exit__(None, None, None)
    ppool_cm = tc.tile_pool(name="psum", bufs=1, space="PSUM")
    ppool = ppool_cm.__enter__()

    # batch-1 x and the out-projection weights are gated behind batch-0's
    # first projection chunk (dummy WAW writes dependent on qkT) so their
    # DMA traffic doesn't compete with the batch-0 load that the phase-1
    # prefix is feed-limited by.
    nc.vector.tensor_copy(xt_sb[:, 1, 0, 0:1], qkT_sb[:, 0, 0:1])
    nc.sync.dma_start(xt_sb[:, 1, :, :], xT[:, 1, :, :])
    nc.vector.tensor_copy(wout_sb[:, 0, 0:1], qkT_sb[:, 0, 0:1])
    nc.sync.dma_start(wout_sb[:], wout[:])

    def p1_units(b, ic_from=0):
        """phase1_seq(b) from chunk ic_from on, decomposed into
        single-matmul emission units so it can be interleaved into a pass2
        as PE filler work. Ordered ic-major (31 units per ic) so a prefix
        makes i-chunk ic of batch b usable."""
        units = []
        state = {}

        def qk_group(ft, ic):
            def alloc():
                state[ft, ic] = ppool.tile([128, 512], F32, tag="mm", bufs=2,
                                           name=f"qk_ps{b}_{ft}_{ic}")
            for dt in range(8):
                def u(ft=ft, ic=ic, dt=dt):
                    if dt == 0:
                        alloc()
                    qk_mm(state[ft, ic], b, ft, ic, dt)
                units.append(u)
            units.append(lambda ft=ft, ic=ic: finish_qk(state[ft, ic], b, ft, ic))

        def v_group(ic):
            def alloc():
                state['v', ic] = ppool.tile([128, 512], F32, tag="mm", bufs=2,
                                            name=f"v_ps{b}_{ic}")
                if ic == ic_from:
                    state['vtb'] = spool.tile([128, N], BF16, tag="vtb",
                                              bufs=2, name=f"vtb{b}")
            for dt in range(8):
                def u(ic=ic, dt=dt):
                    if dt == 0:
                        alloc()
                    vt_mm(state['v', ic], b, ic, dt)
                units.append(u)

            def fin(ic=ic):
                nc.vector.tensor_copy(
                    state['vtb'][:, 512 * ic:512 * (ic + 1)],
                    state['v', ic][:])
            units.append(fin)

        def tr_unit(it):
            def tr(it=it):
                tp = ppool.tile([128, 128], BF16, tag="mm", bufs=2,
                                name=f"t_ps{b}_{it}")
                nc.tensor.transpose(tp[:], state['vtb'][:, 128 * it:128 * (it + 1)],
                                    ident_sb[:])
                nc.vector.tensor_copy(
                    vaug_sb[:, 16 * b + it, :, 64:128],
                    tp[:].rearrange("p (h c) -> p h c", h=HPC))
            units.append(tr)

        for ic in range(ic_from, 4):
            qk_group(0, ic)
            qk_group(1, ic)
            v_group(ic)
            for it in range(4 * ic, 4 * ic + 4):
                tr_unit(it)
        return units

    P1_UNITS_PER_IC = 31

    def proj_units(akey, n_rt, out_b, out_r0):
        """out-projection for 128*n_rt rows from attr_sb[akey]: full w_out
        contraction (8 source K-tiles) into [128,512] psum pairs, as
        filler units. Writes out[out_b, out_r0 + 128*rt ...]."""
        units = []
        state = {}
        for rt in range(n_rt):
            for u in range(8):
                def mm(rt=rt, u=u):
                    if u == 0:
                        for oc in range(2):
                            state[rt, oc] = ppool.tile(
                                [128, 512], F32, tag="mm", bufs=2,
                                name=f"op{akey}_{rt}_{oc}")
                    for oc in range(2):
                        nc.tensor.matmul(
                            state[rt, oc][:],
                            attr_sb[akey][:, u, 128 * rt:128 * (rt + 1)],
                            wout_sb[:, u, 512 * oc:512 * (oc + 1)],
                            start=(u == 0), stop=(u == 7))
                units.append(mm)

            def fin(rt=rt):
                ob = spool.tile([128, D], BF16, tag="ob", bufs=2,
                                name=f"ob{akey}_{rt}")
                for oc in range(2):
                    nc.vector.tensor_copy(ob[:, 512 * oc:512 * (oc + 1)],
                                          state[rt, oc][:])
                nc.sync.dma_start(
                    out[out_b, out_r0 + 128 * rt:out_r0 + 128 * (rt + 1), :],
                    ob[:])
            units.append(fin)
        return units

    last_anf = [None]
    cur_ptp = [None]

    def normalize(b, ic, pvs):
        # denom is pv row 0 (ones row of vaug), per head
        for h in range(HPC):
            if b == 1 and ic == 3:
                # last chunk: nothing follows, and the copy would sit on
                # the exposed A2A#2 trigger chain — normalize from PSUM
                pv = pvs[h, ic]
            else:
                # copy the accumulator out of PSUM first: releases the pv
                # bank immediately so the next chunk's first PV doesn't
                # stall behind the recip/broadcast/mul chain
                praw = spool.tile([128, 512], F32, tag="praw", bufs=4,
                                  name=f"pr{b}_{ic}_{h}")
                nc.vector.tensor_copy(praw[:], pvs[h, ic][:])
                pv = praw
            recip = spool.tile([1, 512], F32, tag="recip", bufs=2,
                               name=f"rc{b}_{ic}_{h}")
            nc.vector.reciprocal_approx_fast(recip[:], pv[0:1, :])
            # partition-broadcast via a DRAM bounce + stride-0-source DMA
            # (SBUF APs reject zero partition stride; DRAM ones don't).
            # NOTHING goes on the GpSimd queue besides the collectives: a
            # gpsimd op between two collectives defers the first one's
            # semaphore increment until the op retires (observed: the
            # A2A#1 gather waiting ~30us past firmware completion behind
            # the chunk-3 broadcast).
            rstage = dpool.tile([1, 512], F32, name=f"rst{b}_{ic}_{h}")
            nc.sync.dma_start(rstage[:], recip[:])
            bc = spool.tile([128, 512], F32, tag="bc", bufs=2,
                            name=f"bc{b}_{ic}_{h}")
            nc.sync.dma_start(bc[64:128, :],
                              rstage[:].to_broadcast((64, 512)))
            bc_sl = bc[64:128, :]
            anf = spool.tile([128, 512], BF16, tag="an", bufs=4,
                             name=f"an{b}_{ic}_{h}")
            nc.vector.tensor_mul(anf[64:128, :], pv[64:128, :], bc_sl)
            last_anf[0] = anf
            # chunk ic covers slots 2ic (cols 0:256) and 2ic+1 (cols
            # 256:512); the slot split is a pure permutation on both sides.
            nc.sync.dma_start(
                a2a_in[b][2 * ic:2 * ic + 2, 64 * h:64 * (h + 1), :]
                .rearrange("s p r -> p s r"),
                anf[64:128, :].rearrange("p (s r) -> p s r", s=2))

    def pass_fused(b, consume=None, at_chunk=None, on_chunk_done=None):
        """causal attention for BOTH heads over all four 512-wide i-chunks
        of batch b, as one software-pipelined stream of j-tiles (the PV of
        tile k is emitted after the S of tile k+1, across chunk
        boundaries). Each j-tile's two heads' S matmuls are concurrent on
        the PE (disjoint row groups). `consume()` emits PE filler work
        once per j-tile; `at_chunk(ic)` runs before each chunk's first
        tile (for prerequisite draining)."""
        plan = [(ic, jt) for ic in range(4) for jt in range(4 * ic + 4)]
        pvs = {}

        def emit_pv(pend):
            pic, pjt, pcp, pptp = pend
            last = (pjt == 4 * pic + 3)
            for h in range(HPC):
                nc.tensor.matmul(pvs[h, pic][:, pcp:512],
                                 vaug_sb[:, 16 * b + pjt, h, :],
                                 pptp[:, h, pcp:512],
                                 start=(pjt == 0), stop=last)
            if last:
                normalize(b, pic, pvs)
                if on_chunk_done is not None:
                    on_chunk_done(pic)

        pend = []
        for ic, jt in plan:
            if jt == 0:
                if at_chunk is not None:
                    at_chunk(ic)
                for h in range(HPC):
                    pvs[h, ic] = ppool.tile([128, 512], F32, tag="pv",
                                            bufs=2, name=f"pv{b}_{ic}_{h}")
            q0 = jt - 4 * ic
            cp = 128 * q0 if q0 > 0 else 0
            sp = ppool.tile([128, 2, 512], F32, tag="s", bufs=2,
                            name=f"s{b}_{ic}_{jt}")
            ptp = ptpool.tile([128, 2, 512], BF16, tag="pt", bufs=16,
                              name=f"pt{b}_{ic}_{jt}")
            for h in range(HPC):
                nc.tensor.matmul(
                    sp[:, h, cp:512],
                    qkT_sb[64 * h:64 * (h + 1), 1,
                           b * N + 128 * jt: b * N + 128 * (jt + 1)],
                    qkT_sb[64 * h:64 * (h + 1), 0,
                           b * N + 512 * ic + cp: b * N + 512 * (ic + 1)],
                    start=True, stop=True)
            nc.scalar.activation(ptp[:, :, cp:512], sp[:, :, cp:512],
                                 AF.Exp, bias=zbias[:], scale=SCALE)
            if q0 >= 0:
                # diagonal tile: only the 128-col diagonal block needs the
                # causal mask (columns right of it are fully valid)
                nc.vector.tensor_mul(ptp[:, :, cp:cp + 128],
                                     ptp[:, :, cp:cp + 128], maskblk_sb[:])
            cur_ptp[0] = ptp
            if consume is not None:
                consume()
            # depth-2 software pipeline: the PV of tile k is emitted after
            # the S of tile k+2, so at chunk boundaries the next chunk's
            # first S/exp outrank the previous chunk's PV backlog in the
            # scheduler's program-order priority.
            if len(pend) >= 2:
                emit_pv(pend.pop(0))
            pend.append((ic, jt, cp, ptp))
        while pend:
            emit_pv(pend.pop(0))

    def do_a2a(key):
        nc.gpsimd.collective_compute(
            "AllToAll", mybir.AluOpType.bypass,
            replica_groups=[list(range(NC))],
            ins=[a2a_in[key].opt()], outs=[a2a_out[key].opt()])
        # gather [8,128,R] -> attr_sb[key] [128, 8, R]. Dispatched from
        # the GpSimd queue: it sits right after its own collective there
        # and fires the moment it completes (on the Sync queue these ended
        # up serialized behind LATER collectives' cumulative thresholds).
        eng = nc.scalar if key == 0 else nc.sync
        for half in range(2):
            eng.dma_start(
                attr_sb[key][:, 4 * half:4 * half + 4, :],
                a2a_out[key][4 * half:4 * half + 4]
                .rearrange("u p r -> p u r"))

    # ---- Phase 2, batch 0 ----
    # batch-0 chunks 2-3 QKV prep and then batch-1 QKV/V prep ride along
    # as PE filler; leftovers drain inside the batch-1 pass (force-drained
    # just in time per chunk).
    units0 = p1_units(0, ic_from=2)
    done0 = [0]
    units1 = p1_units(1)
    done1 = [0]

    def consume_p1(k):
        while k > 0:
            if done0[0] < len(units0):
                units0[done0[0]]()
                done0[0] += 1
            elif done1[0] < len(units1):
                units1[done1[0]]()
                done1[0] += 1
            else:
                break
            k -= 1

    def at_chunk_b0(ic):
        # chunk ic's S matmuls need q/k chunks <= ic and vaug tiles
        # <= 4*ic+3 of batch 0; units0 covers chunks 2-3 ic-major.
        need = P1_UNITS_PER_IC * max(0, ic - 1)
        consume_b0 = max(0, need - done0[0])
        while consume_b0 > 0 and done0[0] < len(units0):
            units0[done0[0]]()
            done0[0] += 1
            consume_b0 -= 1

    pass_fused(0, consume=lambda: consume_p1(3), at_chunk=at_chunk_b0)
    do_a2a(0)

    # ---- Phase 2, batch 1 ----
    # p1 leftovers fill the early tiles. The batch-0 out-projection is NOT
    # consumed in-pass (A_GATE=999): collective completion only becomes
    # engine-visible ~30us after the firmware finishes moving data, so its
    # A2A#1-gated units never actually engage mid-pass — they all drain at
    # the tail, where they land anyway.
    unitsA = proj_units(0, 2, 0, 0)
    doneA = [0]
    jt_ctr = [0]
    A_GATE = 24

    # hold back the last row-tile group of the batch-0 out-projection: it
    # runs DURING the exposed A2A#2 window, keeping the PE p-state warm so
    # the batch-1 out-projection starts at full clock instead of ramping
    A_RESERVE = 9

    def consume_b1():
        jt_ctr[0] += 1
        if done1[0] < len(units1):
            consume_p1(2)
        elif jt_ctr[0] > A_GATE:
            for _ in range(2):
                if doneA[0] < len(unitsA) - A_RESERVE:
                    unitsA[doneA[0]]()
                    doneA[0] += 1

    def at_chunk_b1(ic):
        consume_p1(max(0, P1_UNITS_PER_IC * (ic + 1) - done1[0]))

    # no pre-tail barrier: it CC-serializes ahead of A2A#2, so when cores
    # ARE skewed its own (inflated) duration lands on the critical path —
    # letting A2A#2 absorb the skew directly costs no extra serialization
    pass_fused(1, consume=consume_b1, at_chunk=at_chunk_b1)

    # ---- Phase 3 tail: leftover batch-0 out-projection units, then the
    # A2A#2 (with the reserved units filling its window) + batch-1
    # out-projection + writeback ----
    while doneA[0] < len(unitsA) - A_RESERVE:
        unitsA[doneA[0]]()
        doneA[0] += 1
    do_a2a(1)
    while doneA[0] < len(unitsA):
        unitsA[doneA[0]]()
        doneA[0] += 1
    for u in proj_units(1, 2, 1, 0):
        u()

    for p in reversed(ctx_pools):
        p.__exit__(None, None, None)


def _host_inputs(x, w_qkv, w_out):
    x = np.asarray(x, dtype=np.float32)
    w_qkv = np.asarray(w_qkv, dtype=np.float32)
    w_out = np.asarray(w_out, dtype=np.float32)

    # xT[p, b, dt, i] = x[b, i, 128*dt + p]
    xTt = np.ascontiguousarray(
        x.transpose(2, 0, 1).reshape(8, 128, NB, N).transpose(1, 2, 0, 3)
    ).astype(ml_dtypes.bfloat16)

    wq, wk, wv = w_qkv[:, 0:D], w_qkv[:, D:2 * D], w_qkv[:, 2 * D:3 * D]

    # wout3[p, u, :] = w_out[128*u + p, :]
    wout3 = np.ascontiguousarray(
        w_out.reshape(8, 128, D).transpose(1, 0, 2)).astype(ml_dtypes.bfloat16)

    # diagonal-block causal mask, same for every diagonal j-tile:
    # keep iff (query col within block) >= (key partition)
    k_i = np.arange(128)[:, None]
    c_i = np.arange(128)[None, :]
    mblk = (c_i >= k_i)
    maskblk = np.ascontiguousarray(
        np.stack([mblk, mblk], axis=1)).astype(ml_dtypes.bfloat16)
    identity = np.eye(128, dtype=ml_dtypes.bfloat16)

    in_maps = []
    for c in range(NC):
        sl = slice(FS * c, FS * (c + 1))
        wq_c = np.concatenate([wq[:, sl], wk[:, sl], wv[:, sl]], axis=1)
        wq_c = np.ascontiguousarray(
            wq_c.astype(ml_dtypes.bfloat16).reshape(8, 128, 3 * FS)
            .transpose(1, 0, 2))
        in_maps.append({
            "xT": xTt,
            "wqkv": wq_c,
            "wout": wout3,
            "maskblk": maskblk,
            "ident": identity,
        })
    return in_maps


def run_hw(inputs, trace=False, **kw):
    """Run on 8 NeuronCores. Returns (full_output, BassKernelResults)."""
    global _CACHED_NC
    if _CACHED_NC is None:
        _CACHED_NC = build_graph()
    in_maps = _host_inputs(inputs["x"], inputs["w_qkv"], inputs["w_out"])
    res = run_bass_kernel_spmd(_CACHED_NC, in_maps,
                               core_ids=list(range(NC)), trace=trace, **kw)
    # core c's out is [NB, 256, D] = rows [256c, 256c+256) of each batch
    y = np.concatenate([np.asarray(res.results[c]["out"]) for c in range(NC)],
                       axis=1).astype(np.float32)
    return y, res


def kernel(**inputs):
    y, _ = run_hw(inputs, trace=bool(os.environ.get("BASS_TRACE")))
    return y


# revision 83
# speedup vs baseline: 1.1035x; 1.0049x over previous
"""Distributed causal multi-head attention for Trainium2 (8 NeuronCores).

Problem: x[2,2048,1024] @ w_qkv[1024,3072] -> 16-head causal attention
         -> @ w_out[1024,1024]. fp32 reference; device compute in bf16
         (fp32 PSUM accumulation).

Sharding (8 cores): core c owns heads {2c, 2c+1} for BOTH batches
(feature slice 128c..128c+128 of the qkv projections). Output rows are
sharded batch-major: core c owns rows [256c, 256c+256) of each batch.

Phase 2 runs one fused pass per BATCH: the two heads' S matmuls are
K=64 and live on disjoint PE row groups (head A at partitions 0:64 ->
tile rows 0:64, head B at 64:128 -> rows 64:128, tile_position derived
from base partitions), so emitting them back-to-back lets the PE
execute them concurrently (~2x on the score matmuls). The AllToAll is
split per batch ([8 slots, 128 dims, 256 rows] each): A2A#1 (batch 0)
overlaps the batch-1 pass; only A2A#2 is exposed at the tail.

Scheduling notes (the Tile scheduler reorders by readiness, and
semaphore wait thresholds are pinned at the simulated positions):
  - barriers are tiny AllToAlls, not AllGathers — mixing collective
    kinds desynchronized the Collectives-semaphore thresholds on hw
  - a sacrificial barrier absorbs the ~11us CC-firmware first-op ramp;
    no pre-tail barrier (it CC-serializes ahead of A2A#2, putting its
    own skew-inflated duration on the critical path)
  - normalize's partition broadcast uses a DRAM-bounce stride-0 DMA
    (GpSimd blocks behind in-flight collective_compute triggers)
  - the PV accumulators are copied out of PSUM before normalization so
    the banks recycle without waiting on the recip/broadcast chain
  - phase 1 runs only chunks 0-1 before the batch-0 pass (DMA-feed
    paced); chunks 2-3 and all of batch 1 are consume()-paced PE
    filler inside the passes, force-drained per chunk just in time

Device pipeline per core:
  P1: qT,kT = (w_qk stationary) @ xT chunks   [bf16, N=512 moving]
      vT    = (w_v stationary)  @ xT chunks -> PE-transpose -> V seq-major
      vaug  = [ones | pad | V_h] per j-tile   [ones row 0 => denominators]
  P2 (per batch b, i-chunk of 512, j-tile of 128): depth-2 software
      pipeline: S^T[j,i] for BOTH heads (two concurrent K=64 matmuls
      into one [128,2,512] PSUM tile) -> one ACT exp (scale fused,
      bf16) -> 128-col diagonal-block mask mul (DVE) -> per-head PV
      accumulate (row 0 = denominators) -> normalize -> DMA into the
      batch A2A buffer (slot split via pure-permutation APs).
  P3: per batch: AllToAll [8,128,256] bf16; gather to attr_sb; out
      rows = sum over 8 source K-tiles (full w_out contraction in one
      round, no partial staging); bf16 writeback.
"""
import os
import numpy as np
import ml_dtypes

import concourse.bass as bass
import concourse.bacc as bacc
import concourse.mybir as mybir
import concourse.tile as tile
from concourse.bass_utils import run_bass_kernel_spmd

F32 = mybir.dt.float32
BF16 = mybir.dt.bfloat16
AF = mybir.ActivationFunctionType

NC = 8           # cores
NB = 2           # batches
N = 2048         # seq len
D = 1024         # model dim
HPC = 2          # heads per core
HD = 64          # head dim
FS = HPC * HD    # per-core feature slice (128)
NFLAT = NB * N   # 4096 flattened rows
RPS = N // NC    # 256 rows per A2A slot (per batch)
SCALE = HD ** -0.5

_CACHED_NC = None


def build_graph():
    nc = bacc.Bacc("TRN2", target_bir_lowering=False, debug=False,
                   num_devices=NC)

    xT = nc.dram_tensor("xT", [128, NB, 8, N], BF16, kind="ExternalInput")
    wqkv = nc.dram_tensor("wqkv", [128, 8, 3 * FS], BF16, kind="ExternalInput")
    wout = nc.dram_tensor("wout", [128, 8, D], BF16, kind="ExternalInput")
    maskblk = nc.dram_tensor("maskblk", [128, 2, 128], BF16,
                             kind="ExternalInput")
    ident = nc.dram_tensor("ident", [128, 128], BF16, kind="ExternalInput")
    out = nc.dram_tensor("out", [NB, RPS, D], BF16, kind="ExternalOutput")

    with tile.TileContext(nc) as tc:
        _emit(nc, tc, xT, wqkv, wout, maskblk, ident, out)
    nc.compile()
    return nc


def _emit(nc, tc, xT, wqkv, wout, maskblk, ident, out):
    ctx_pools = []

    def pool(name, **kw):
        cm = tc.tile_pool(name=name, **kw)
        p = cm.__enter__()
        ctx_pools.append(cm)
        return p

    wpool = pool("weights", bufs=1)
    ptpool = pool("pt", bufs=8)
    spool = pool("stage", bufs=1)
    dpool = pool("dram", bufs=1, space="DRAM")
    pinit_cm = tc.tile_pool(name="psum_init", bufs=1, space="PSUM")
    pinit = pinit_cm.__enter__()

    # ---- persistent SBUF buffers ----
    xt_sb = wpool.tile([128, NB, 8, N], BF16)
    wqkv_sb = wpool.tile([128, 8, 3 * FS], BF16)
    wout_sb = wpool.tile([128, 8, D], BF16)
    maskblk_sb = wpool.tile([128, 2, 128], BF16)
    ident_sb = wpool.tile([128, 128], BF16)
    qkT_sb = wpool.tile([128, 2, NFLAT], BF16)          # [dims, q/k, b*N+i]
    # per j-tile [ones | junk | V_h]: row0=ones, rows 64:128 = V dims
    vaug_sb = wpool.tile([128, 32, HPC, 128], BF16)
    attr_sb = {b: wpool.tile([128, NC, RPS], BF16, name=f"attr{b}")
               for b in range(NB)}

    a2a_in = {b: dpool.tile([NC, FS, RPS], BF16, name=f"a2ai{b}")
              for b in range(NB)}
    a2a_out = {b: dpool.tile([NC, FS, RPS], BF16, name=f"a2ao{b}")
               for b in range(NB)}
    # barriers are implemented as tiny AllToAlls, NOT AllGathers: on this
    # runtime only AllToAll completions advance the Collectives semaphore,
    # so mixing kinds desynchronizes the tile framework's cumulative wait
    # thresholds (observed: the A2A#1 gather waiting on A2A#2).
    bar_in = dpool.tile([NC, 16], F32, name="bar_in")
    bar_out = dpool.tile([NC, 16], F32, name="bar_out")
    bar_in2 = dpool.tile([NC, 16], F32, name="bar_in2")
    bar_out2 = dpool.tile([NC, 16], F32, name="bar_out2")

    # startup DMAs: few LARGE transfers (big per-partition contiguous
    # descriptors — 4KB descriptors only reach ~half DMA throughput).
    nc.sync.dma_start(wqkv_sb[:, 0:2, :], wqkv[:, 0:2, :])
    nc.sync.dma_start(xt_sb[:, 0, 0:2, :], xT[:, 0, 0:2, :])
    nc.sync.dma_start(wqkv_sb[:, 2:8, :], wqkv[:, 2:8, :])
    nc.sync.dma_start(ident_sb[:], ident[:])
    for q in range(1, 4):
        nc.sync.dma_start(xt_sb[:, 0, 2 * q:2 * q + 2, :],
                          xT[:, 0, 2 * q:2 * q + 2, :])
    nc.sync.dma_start(maskblk_sb[:], maskblk[:])
    nc.vector.memset(vaug_sb[:, :, :, 0:1], 1.0)
    nc.vector.memset(vaug_sb[:, :, :, 1:64], 0.0)
    # explicit zero bias for Exp: avoids the shared const-0.0 SBUF tensor,
    # whose region aliases later pool tiles and trips false DMA/ACT races
    zbias = wpool.tile([128, 1], F32, name="zbias")
    nc.vector.memset(zbias[:], 0.0)
    bar_sb = wpool.tile([NC, 16], F32, name="bar_sb")
    nc.vector.memset(bar_sb[:], 0.0)
    nc.sync.dma_start(bar_in[:], bar_sb[:])
    # sacrificial barrier, ready immediately: the first collective after
    # the NEFF init barrier pays an ~11us CC-firmware ramp — let this one
    # absorb it during phase 1 instead of A2A#1.
    nc.gpsimd.collective_compute(
        "AllToAll", mybir.AluOpType.bypass,
        replica_groups=[list(range(NC))],
        ins=[bar_in.opt()], outs=[bar_out.opt()])

    def qk_mm(ps, b, ft, ic, dt):
        nc.tensor.matmul(
            ps[:],
            wqkv_sb[:, dt, 128 * ft:128 * (ft + 1)],
            xt_sb[:, b, dt, 512 * ic:512 * (ic + 1)],
            start=(dt == 0), stop=(dt == 7))

    def vt_mm(ps, b, ic, dt):
        nc.tensor.matmul(
            ps[:],
            wqkv_sb[:, dt, 2 * FS:3 * FS],
            xt_sb[:, b, dt, 512 * ic:512 * (ic + 1)],
            start=(dt == 0), stop=(dt == 7))

    def finish_qk(ps, b, ft, ic):
        nc.vector.tensor_copy(
            qkT_sb[:, ft, b * N + 512 * ic: b * N + 512 * (ic + 1)], ps[:])

    def finish_v(vps_list, b, psum_pool, ptag, pbufs):
        vT_bf = spool.tile([128, N], BF16, tag="vtb", bufs=2, name=f"vtb{b}")
        for ic in range(4):
            nc.vector.tensor_copy(vT_bf[:, 512 * ic:512 * (ic + 1)],
                                  vps_list[ic][:])
        for it in range(16):
            tp = psum_pool.tile([128, 128], BF16, tag=ptag, bufs=pbufs,
                                name=f"t_ps{b}_{it}")
            nc.tensor.transpose(tp[:], vT_bf[:, 128 * it:128 * (it + 1)],
                                ident_sb[:])
            nc.vector.tensor_copy(
                vaug_sb[:, 16 * b + it, :, 64:128],
                tp[:].rearrange("p (h c) -> p h c", h=HPC))

    # ---- warmup while the xT DMA streams in ----
    # preload the ACT exp table (first use costs ~1.3us)
    wsc = spool.tile([128, 1], BF16, tag="wsc", name="wsc")
    nc.scalar.activation(wsc[:], zbias[:], AF.Exp, bias=zbias[:], scale=1.0)

    # ---- Phase 1, batch 0, chunks 0-1 only: dt-outer passes, paced by
    # the xT DMA feed. Chunks 2-3 become pass-0 filler units so the
    # attention pass starts ~20us earlier. ----
    qk_ps = {(ft, ic): pinit.tile([128, 512], F32, tag="init",
                                  bufs=8, name=f"qk0_{ft}_{ic}")
             for ft in range(2) for ic in range(2)}
    v_ps0 = {ic: pinit.tile([128, 512], F32, tag="init", bufs=8,
                            name=f"v0_{ic}") for ic in range(2)}
    for dt in range(8):
        for ft in range(2):
            for ic in range(2):
                qk_mm(qk_ps[ft, ic], 0, ft, ic, dt)
        for ic in range(2):
            vt_mm(v_ps0[ic], 0, ic, dt)
    for ft in range(2):
        for ic in range(2):
            finish_qk(qk_ps[ft, ic], 0, ft, ic)
    vT_bf0 = spool.tile([128, N], BF16, tag="vtb", bufs=2, name="vtb0")
    for ic in range(2):
        nc.vector.tensor_copy(vT_bf0[:, 512 * ic:512 * (ic + 1)],
                              v_ps0[ic][:])
    for it in range(8):
        tp = pinit.tile([128, 128], BF16, tag="init", bufs=8,
                        name=f"t_ps0_{it}")
        nc.tensor.transpose(tp[:], vT_bf0[:, 128 * it:128 * (it + 1)],
                            ident_sb[:])
        nc.vector.tensor_copy(
            vaug_sb[:, it, :, 64:128],
            tp[:].rearrange("p (h c) -> p h c", h=HPC))
    pinit_cm.__exit__(None, None, None)
    ppool_cm = tc.tile_pool(name="psum", bufs=1, space="PSUM")
    ppool = ppool_cm.__enter__()

    # batch-1 x and the out-projection weights are gated behind batch-0's
    # first projection chunk (dummy WAW writes dependent on qkT) so their
    # DMA traffic doesn't compete with the batch-0 load that the phase-1
    # prefix is feed-limited by.
    nc.vector.tensor_copy(xt_sb[:, 1, 0, 0:1], qkT_sb[:, 0, 0:1])
    nc.sync.dma_start(xt_sb[:, 1, :, :], xT[:, 1, :, :])
    nc.vector.tensor_copy(wout_sb[:, 0, 0:1], qkT_sb[:, 0, 0:1])
    nc.sync.dma_start(wout_sb[:], wout[:])

    def p1_units(b, ic_from=0):
        """phase1_seq(b) from chunk ic_from on, decomposed into
        single-matmul emission units so it can be interleaved into a pass2
        as PE filler work. Ordered ic-major (31 units per ic) so a prefix
        makes i-chunk ic of batch b usable."""
        units = []
        state = {}

        def qk_group(ft, ic):
            def alloc():
                state[ft, ic] = ppool.tile([128, 512], F32, tag="mm", bufs=2,
                                           name=f"qk_ps{b}_{ft}_{ic}")
            for dt in range(8):
                def u(ft=ft, ic=ic, dt=dt):
                    if dt == 0:
                        alloc()
                    qk_mm(state[ft, ic], b, ft, ic, dt)
                units.append(u)
            units.append(lambda ft=ft, ic=ic: finish_qk(state[ft, ic], b, ft, ic))

        def v_group(ic):
            def alloc():
                state['v', ic] = ppool.tile([128, 512], F32, tag="mm", bufs=2,
                                            name=f"v_ps{b}_{ic}")
                if ic == ic_from:
                    state['vtb'] = spool.tile([128, N], BF16, tag="vtb",
                                              bufs=2, name=f"vtb{b}")
            for dt in range(8):
                def u(ic=ic, dt=dt):
                    if dt == 0:
                        alloc()
                    vt_mm(state['v', ic], b, ic, dt)
                units.append(u)

            def fin(ic=ic):
                nc.vector.tensor_copy(
                    state['vtb'][:, 512 * ic:512 * (ic + 1)],
                    state['v', ic][:])
            units.append(fin)

        def tr_unit(it):
            def tr(it=it):
                tp = ppool.tile([128, 128], BF16, tag="mm", bufs=2,
                                name=f"t_ps{b}_{it}")
                nc.tensor.transpose(tp[:], state['vtb'][:, 128 * it:128 * (it + 1)],
                                    ident_sb[:])
                nc.vector.tensor_copy(
                    vaug_sb[:, 16 * b + it, :, 64:128],
                    tp[:].rearrange("p (h c) -> p h c", h=HPC))
            units.append(tr)

        for ic in range(ic_from, 4):
            qk_group(0, ic)
            qk_group(1, ic)
            v_group(ic)
            for it in range(4 * ic, 4 * ic + 4):
                tr_unit(it)
        return units

    P1_UNITS_PER_IC = 31

    def proj_units(akey, n_rt, out_b, out_r0):
        """out-projection for 128*n_rt rows from attr_sb[akey]: full w_out
        contraction (8 source K-tiles) into [128,512] psum pairs, as
        filler units. Writes out[out_b, out_r0 + 128*rt ...]."""
        units = []
        state = {}
        for rt in range(n_rt):
            for u in range(8):
                def mm(rt=rt, u=u):
                    if u == 0:
                        for oc in range(2):
                            state[rt, oc] = ppool.tile(
                                [128, 512], F32, tag="mm", bufs=2,
                                name=f"op{akey}_{rt}_{oc}")
                    for oc in range(2):
                        nc.tensor.matmul(
                            state[rt, oc][:],
                            attr_sb[akey][:, u, 128 * rt:128 * (rt + 1)],
                            wout_sb[:, u, 512 * oc:512 * (oc + 1)],
                            start=(u == 0), stop=(u == 7))
                units.append(mm)

            def fin(rt=rt):
                ob = spool.tile([128, D], BF16, tag="ob", bufs=2,
                                name=f"ob{akey}_{rt}")
                for oc in range(2):
                    nc.vector.tensor_copy(ob[:, 512 * oc:512 * (oc + 1)],
                                          state[rt, oc][:])
                nc.sync.dma_start(
                    out[out_b, out_r0 + 128 * rt:out_r0 + 128 * (rt + 1), :],
                    ob[:])
            units.append(fin)
        return units

    last_anf = [None]
    cur_ptp = [None]

    def normalize(b, ic, pvs):
        # denom is pv row 0 (ones row of vaug), per head
        for h in range(HPC):
            if b == 1 and ic == 3:
                # last chunk: nothing follows, and the copy would sit on
                # the exposed A2A#2 trigger chain — normalize from PSUM
                pv = pvs[h, ic]
            else:
                # copy the accumulator out of PSUM first: releases the pv
                # bank immediately so the next chunk's first PV doesn't
                # stall behind the recip/broadcast/mul chain
                praw = spool.tile([128, 512], F32, tag="praw", bufs=4,
                                  name=f"pr{b}_{ic}_{h}")
                nc.vector.tensor_copy(praw[:], pvs[h, ic][:])
                pv = praw
            recip = spool.tile([1, 512], F32, tag="recip", bufs=2,
                               name=f"rc{b}_{ic}_{h}")
            nc.vector.reciprocal_approx_fast(recip[:], pv[0:1, :])
            # partition-broadcast via a DRAM bounce + stride-0-source DMA
            # (SBUF APs reject zero partition stride; DRAM ones don't).
            # NOTHING goes on the GpSimd queue besides the collectives: a
            # gpsimd op between two collectives defers the first one's
            # semaphore increment until the op retires (observed: the
            # A2A#1 gather waiting ~30us past firmware completion behind
            # the chunk-3 broadcast).
            rstage = dpool.tile([1, 512], F32, name=f"rst{b}_{ic}_{h}")
            nc.sync.dma_start(rstage[:], recip[:])
            bc = spool.tile([128, 512], F32, tag="bc", bufs=2,
                            name=f"bc{b}_{ic}_{h}")
            nc.sync.dma_start(bc[64:128, :],
                              rstage[:].to_broadcast((64, 512)))
            bc_sl = bc[64:128, :]
            anf = spool.tile([128, 512], BF16, tag="an", bufs=4,
                             name=f"an{b}_{ic}_{h}")
            nc.vector.tensor_mul(anf[64:128, :], pv[64:128, :], bc_sl)
            last_anf[0] = anf
            # chunk ic covers slots 2ic (cols 0:256) and 2ic+1 (cols
            # 256:512); the slot split is a pure permutation on both sides.
            nc.sync.dma_start(
                a2a_in[b][2 * ic:2 * ic + 2, 64 * h:64 * (h + 1), :]
                .rearrange("s p r -> p s r"),
                anf[64:128, :].rearrange("p (s r) -> p s r", s=2))

    def pass_fused(b, consume=None, at_chunk=None, on_chunk_done=None):
        """causal attention for BOTH heads over all four 512-wide i-chunks
        of batch b, as one software-pipelined stream of j-tiles (the PV of
        tile k is emitted after the S of tile k+1, across chunk
        boundaries). Each j-tile's two heads' S matmuls are concurrent on
        the PE (disjoint row groups). `consume()` emits PE filler work
        once per j-tile; `at_chunk(ic)` runs before each chunk's first
        tile (for prerequisite draining)."""
        plan = [(ic, jt) for ic in range(4) for jt in range(4 * ic + 4)]
        pvs = {}

        def emit_pv(pend):
            pic, pjt, pcp, pptp = pend
            last = (pjt == 4 * pic + 3)
            for h in range(HPC):
                nc.tensor.matmul(pvs[h, pic][:, pcp:512],
                                 vaug_sb[:, 16 * b + pjt, h, :],
                                 pptp[:, h, pcp:512],
                                 start=(pjt == 0), stop=last)
            if last:
                normalize(b, pic, pvs)
                if on_chunk_done is not None:
                    on_chunk_done(pic)

        pend = []
        for ic, jt in plan:
            if jt == 0:
                if at_chunk is not None:
                    at_chunk(ic)
                for h in range(HPC):
                    pvs[h, ic] = ppool.tile([128, 512], F32, tag="pv",
                                            bufs=2, name=f"pv{b}_{ic}_{h}")
            q0 = jt - 4 * ic
            cp = 128 * q0 if q0 > 0 else 0
            sp = ppool.tile([128, 2, 512], F32, tag="s", bufs=2,
                            name=f"s{b}_{ic}_{jt}")
            ptp = ptpool.tile([128, 2, 512], BF16, tag="pt", bufs=24,
                              name=f"pt{b}_{ic}_{jt}")
            for h in range(HPC):
                nc.tensor.matmul(
                    sp[:, h, cp:512],
                    qkT_sb[64 * h:64 * (h + 1), 1,
                           b * N + 128 * jt: b * N + 128 * (jt + 1)],
                    qkT_sb[64 * h:64 * (h + 1), 0,
                           b * N + 512 * ic + cp: b * N + 512 * (ic + 1)],
                    start=True, stop=True)
            nc.scalar.activation(ptp[:, :, cp:512], sp[:, :, cp:512],
                                 AF.Exp, bias=zbias[:], scale=SCALE)
            if q0 >= 0:
                # diagonal tile: only the 128-col diagonal block needs the
                # causal mask (columns right of it are fully valid)
                nc.vector.tensor_mul(ptp[:, :, cp:cp + 128],
                                     ptp[:, :, cp:cp + 128], maskblk_sb[:])
            cur_ptp[0] = ptp
            if consume is not None:
                consume()
            # depth-2 software pipeline: the PV of tile k is emitted after
            # the S of tile k+2, so at chunk boundaries the next chunk's
            # first S/exp outrank the previous chunk's PV backlog in the
            # scheduler's program-order priority.
            if len(pend) >= 2:
                emit_pv(pend.pop(0))
            pend.append((ic, jt, cp, ptp))
        while pend:
            emit_pv(pend.pop(0))

    def do_a2a(key):
        nc.gpsimd.collective_compute(
            "AllToAll", mybir.AluOpType.bypass,
            replica_groups=[list(range(NC))],
            ins=[a2a_in[key].opt()], outs=[a2a_out[key].opt()])
        # gather [8,128,R] -> attr_sb[key] [128, 8, R]. Dispatched from
        # the GpSimd queue: it sits right after its own collective there
        # and fires the moment it completes (on the Sync queue these ended
        # up serialized behind LATER collectives' cumulative thresholds).
        eng = nc.scalar if key == 0 else nc.sync
        for half in range(2):
            eng.dma_start(
                attr_sb[key][:, 4 * half:4 * half + 4, :],
                a2a_out[key][4 * half:4 * half + 4]
                .rearrange("u p r -> p u r"))

    # ---- Phase 2, batch 0 ----
    # batch-0 chunks 2-3 QKV prep and then batch-1 QKV/V prep ride along
    # as PE filler; leftovers drain inside the batch-1 pass (force-drained
    # just in time per chunk).
    units0 = p1_units(0, ic_from=2)
    done0 = [0]
    units1 = p1_units(1)
    done1 = [0]

    def consume_p1(k):
        while k > 0:
            if done0[0] < len(units0):
                units0[done0[0]]()
                done0[0] += 1
            elif done1[0] < len(units1):
                units1[done1[0]]()
                done1[0] += 1
            else:
                break
            k -= 1

    def at_chunk_b0(ic):
        # chunk ic's S matmuls need q/k chunks <= ic and vaug tiles
        # <= 4*ic+3 of batch 0; units0 covers chunks 2-3 ic-major.
        need = P1_UNITS_PER_IC * max(0, ic - 1)
        consume_b0 = max(0, need - done0[0])
        while consume_b0 > 0 and done0[0] < len(units0):
            units0[done0[0]]()
            done0[0] += 1
            consume_b0 -= 1

    pass_fused(0, consume=lambda: consume_p1(3), at_chunk=at_chunk_b0)
    do_a2a(0)

    # ---- Phase 2, batch 1 ----
    # p1 leftovers fill the early tiles. The batch-0 out-projection is NOT
    # consumed in-pass (A_GATE=999): collective completion only becomes
    # engine-visible ~30us after the firmware finishes moving data, so its
    # A2A#1-gated units never actually engage mid-pass — they all drain at
    # the tail, where they land anyway.
    unitsA = proj_units(0, 2, 0, 0)
    doneA = [0]
    jt_ctr = [0]
    A_GATE = 24

    # hold back the last row-tile group of the batch-0 out-projection: it
    # runs DURING the exposed A2A#2 window, keeping the PE p-state warm so
    # the batch-1 out-projection starts at full clock instead of ramping
    A_RESERVE = 9

    def consume_b1():
        jt_ctr[0] += 1
        if done1[0] < len(units1):
            consume_p1(2)
        elif jt_ctr[0] > A_GATE:
            for _ in range(2):
                if doneA[0] < len(unitsA) - A_RESERVE:
                    unitsA[doneA[0]]()
                    doneA[0] += 1

    def at_chunk_b1(ic):
        consume_p1(max(0, P1_UNITS_PER_IC * (ic + 1) - done1[0]))

    # no pre-tail barrier: it CC-serializes ahead of A2A#2, so when cores
    # ARE skewed its own (inflated) duration lands on the critical path —
    # letting A2A#2 absorb the skew directly costs no extra serialization
    pass_fused(1, consume=consume_b1, at_chunk=at_chunk_b1)

    # ---- Phase 3 tail: leftover batch-0 out-projection units, then the
    # A2A#2 (with the reserved units filling its window) + batch-1
    # out-projection + writeback ----
    while doneA[0] < len(unitsA) - A_RESERVE:
        unitsA[doneA[0]]()
        doneA[0] += 1
    do_a2a(1)
    while doneA[0] < len(unitsA):
        unitsA[doneA[0]]()
        doneA[0] += 1
    for u in proj_units(1, 2, 1, 0):
        u()

    for p in reversed(ctx_pools):
        p.__exit__(None, None, None)


def _host_inputs(x, w_qkv, w_out):
    x = np.asarray(x, dtype=np.float32)
    w_qkv = np.asarray(w_qkv, dtype=np.float32)
    w_out = np.asarray(w_out, dtype=np.float32)

    # xT[p, b, dt, i] = x[b, i, 128*dt + p]
    xTt = np.ascontiguousarray(
        x.transpose(2, 0, 1).reshape(8, 128, NB, N).transpose(1, 2, 0, 3)
    ).astype(ml_dtypes.bfloat16)

    wq, wk, wv = w_qkv[:, 0:D], w_qkv[:, D:2 * D], w_qkv[:, 2 * D:3 * D]

    # wout3[p, u, :] = w_out[128*u + p, :]
    wout3 = np.ascontiguousarray(
        w_out.reshape(8, 128, D).transpose(1, 0, 2)).astype(ml_dtypes.bfloat16)

    # diagonal-block causal mask, same for every diagonal j-tile:
    # keep iff (query col within block) >= (key partition)
    k_i = np.arange(128)[:, None]
    c_i = np.arange(128)[None, :]
    mblk = (c_i >= k_i)
    maskblk = np.ascontiguousarray(
        np.stack([mblk, mblk], axis=1)).astype(ml_dtypes.bfloat16)
    identity = np.eye(128, dtype=ml_dtypes.bfloat16)

    in_maps = []
    for c in range(NC):
        sl = slice(FS * c, FS * (c + 1))
        wq_c = np.concatenate([wq[:, sl], wk[:, sl], wv[:, sl]], axis=1)
        wq_c = np.ascontiguousarray(
            wq_c.astype(ml_dtypes.bfloat16).reshape(8, 128, 3 * FS)
            .transpose(1, 0, 2))
        in_maps.append({
            "xT": xTt,
            "wqkv": wq_c,
            "wout": wout3,
            "maskblk": maskblk,
            "ident": identity,
        })
    return in_maps


def run_hw(inputs, trace=False, **kw):
    """Run on 8 NeuronCores. Returns (full_output, BassKernelResults)."""
    global _CACHED_NC
    if _CACHED_NC is None:
        _CACHED_NC = build_graph()
    in_maps = _host_inputs(inputs["x"], inputs["w_qkv"], inputs["w_out"])
    res = run_bass_kernel_spmd(_CACHED_NC, in_maps,
                               core_ids=list(range(NC)), trace=trace, **kw)
    # core c's out is [NB, 256, D] = rows [256c, 256c+256) of each batch
    y = np.concatenate([np.asarray(res.results[c]["out"]) for c in range(NC)],
                       axis=1).astype(np.float32)
    return y, res


def kernel(**inputs):
    y, _ = run_hw(inputs, trace=bool(os.environ.get("BASS_TRACE")))
    return y


# revision 84
# speedup vs baseline: 1.1337x; 1.0274x over previous
"""Distributed causal multi-head attention for Trainium2 (8 NeuronCores).

Problem: x[2,2048,1024] @ w_qkv[1024,3072] -> 16-head causal attention
         -> @ w_out[1024,1024]. fp32 reference; device compute in bf16
         (fp32 PSUM accumulation).

Sharding (8 cores): core c owns heads {2c, 2c+1} for BOTH batches
(feature slice 128c..128c+128 of the qkv projections). Output rows are
sharded batch-major: core c owns rows [256c, 256c+256) of each batch.

Phase 2 runs one fused pass per BATCH: the two heads' S matmuls are
K=64 and live on disjoint PE row groups (head A at partitions 0:64 ->
tile rows 0:64, head B at 64:128 -> rows 64:128, tile_position derived
from base partitions), so emitting them back-to-back lets the PE
execute them concurrently (~2x on the score matmuls). The AllToAll is
split per batch ([8 slots, 128 dims, 256 rows] each): A2A#1 (batch 0)
overlaps the batch-1 pass; only A2A#2 is exposed at the tail.

Scheduling notes (the Tile scheduler reorders by readiness, and
semaphore wait thresholds are pinned at the simulated positions):
  - barriers are tiny AllToAlls, not AllGathers — mixing collective
    kinds desynchronized the Collectives-semaphore thresholds on hw
  - a sacrificial barrier absorbs the ~11us CC-firmware first-op ramp;
    no pre-tail barrier (it CC-serializes ahead of A2A#2, putting its
    own skew-inflated duration on the critical path)
  - normalize's partition broadcast uses a DRAM-bounce stride-0 DMA
    (GpSimd blocks behind in-flight collective_compute triggers)
  - the PV accumulators are copied out of PSUM before normalization so
    the banks recycle without waiting on the recip/broadcast chain
  - phase 1 runs only chunks 0-1 before the batch-0 pass (DMA-feed
    paced); chunks 2-3 and all of batch 1 are consume()-paced PE
    filler inside the passes, force-drained per chunk just in time

Device pipeline per core:
  P1: qT,kT = (w_qk stationary) @ xT chunks   [bf16, N=512 moving]
      vT    = (w_v stationary)  @ xT chunks -> PE-transpose -> V seq-major
      vaug  = [ones | pad | V_h] per j-tile   [ones row 0 => denominators]
  P2 (per batch b, i-chunk of 512, j-tile of 128): depth-2 software
      pipeline: S^T[j,i] for BOTH heads (two concurrent K=64 matmuls
      into one [128,2,512] PSUM tile) -> one ACT exp (scale fused,
      bf16) -> 128-col diagonal-block mask mul (DVE) -> per-head PV
      accumulate (row 0 = denominators) -> normalize -> DMA into the
      batch A2A buffer (slot split via pure-permutation APs).
  P3: per batch: AllToAll [8,128,256] bf16; gather to attr_sb; out
      rows = sum over 8 source K-tiles (full w_out contraction in one
      round, no partial staging); bf16 writeback.
"""
import os
import numpy as np
import ml_dtypes

import concourse.bass as bass
import concourse.bacc as bacc
import concourse.mybir as mybir
import concourse.tile as tile
from concourse.bass_utils import run_bass_kernel_spmd

F32 = mybir.dt.float32
BF16 = mybir.dt.bfloat16
AF = mybir.ActivationFunctionType

NC = 8           # cores
NB = 2           # batches
N = 2048         # seq len
D = 1024         # model dim
HPC = 2          # heads per core
HD = 64          # head dim
FS = HPC * HD    # per-core feature slice (128)
NFLAT = NB * N   # 4096 flattened rows
RPS = N // NC    # 256 rows per A2A slot (per batch)
SCALE = HD ** -0.5

_CACHED_NC = None


def build_graph():
    nc = bacc.Bacc("TRN2", target_bir_lowering=False, debug=False,
                   num_devices=NC)

    xT = nc.dram_tensor("xT", [128, NB, 8, N], BF16, kind="ExternalInput")
    wqkv = nc.dram_tensor("wqkv", [128, 8, 3 * FS], BF16, kind="ExternalInput")
    wout = nc.dram_tensor("wout", [128, 8, D], BF16, kind="ExternalInput")
    maskblk = nc.dram_tensor("maskblk", [128, 2, 128], BF16,
                             kind="ExternalInput")
    ident = nc.dram_tensor("ident", [128, 128], BF16, kind="ExternalInput")
    out = nc.dram_tensor("out", [NB, RPS, D], BF16, kind="ExternalOutput")

    with tile.TileContext(nc) as tc:
        _emit(nc, tc, xT, wqkv, wout, maskblk, ident, out)
    nc.compile()
    return nc


def _emit(nc, tc, xT, wqkv, wout, maskblk, ident, out):
    ctx_pools = []

    def pool(name, **kw):
        cm = tc.tile_pool(name=name, **kw)
        p = cm.__enter__()
        ctx_pools.append(cm)
        return p

    wpool = pool("weights", bufs=1)
    ptpool = pool("pt", bufs=8)
    spool = pool("stage", bufs=1)
    dpool = pool("dram", bufs=1, space="DRAM")
    pinit_cm = tc.tile_pool(name="psum_init", bufs=1, space="PSUM")
    pinit = pinit_cm.__enter__()

    # ---- persistent SBUF buffers ----
    xt_sb = wpool.tile([128, NB, 8, N], BF16)
    wqkv_sb = wpool.tile([128, 8, 3 * FS], BF16)
    wout_sb = wpool.tile([128, 8, D], BF16)
    maskblk_sb = wpool.tile([128, 2, 128], BF16)
    ident_sb = wpool.tile([128, 128], BF16)
    qkT_sb = wpool.tile([128, 2, NFLAT], BF16)          # [dims, q/k, b*N+i]
    # per j-tile [ones | junk | V_h]: row0=ones, rows 64:128 = V dims
    vaug_sb = wpool.tile([128, 32, HPC, 128], BF16)
    attr_sb = {b: wpool.tile([128, NC, RPS], BF16, name=f"attr{b}")
               for b in range(NB)}

    a2a_in = {b: dpool.tile([NC, FS, RPS], BF16, name=f"a2ai{b}")
              for b in range(NB)}
    a2a_out = {b: dpool.tile([NC, FS, RPS], BF16, name=f"a2ao{b}")
               for b in range(NB)}
    # barriers are implemented as tiny AllToAlls, NOT AllGathers: on this
    # runtime only AllToAll completions advance the Collectives semaphore,
    # so mixing kinds desynchronizes the tile framework's cumulative wait
    # thresholds (observed: the A2A#1 gather waiting on A2A#2).
    bar_in = dpool.tile([NC, 16], F32, name="bar_in")
    bar_out = dpool.tile([NC, 16], F32, name="bar_out")
    bar_in2 = dpool.tile([NC, 16], F32, name="bar_in2")
    bar_out2 = dpool.tile([NC, 16], F32, name="bar_out2")

    # startup DMAs: few LARGE transfers (big per-partition contiguous
    # descriptors — 4KB descriptors only reach ~half DMA throughput).
    nc.sync.dma_start(wqkv_sb[:, 0:2, :], wqkv[:, 0:2, :])
    nc.sync.dma_start(xt_sb[:, 0, 0:2, :], xT[:, 0, 0:2, :])
    nc.sync.dma_start(wqkv_sb[:, 2:8, :], wqkv[:, 2:8, :])
    nc.sync.dma_start(ident_sb[:], ident[:])
    for q in range(1, 4):
        nc.sync.dma_start(xt_sb[:, 0, 2 * q:2 * q + 2, :],
                          xT[:, 0, 2 * q:2 * q + 2, :])
    nc.sync.dma_start(maskblk_sb[:], maskblk[:])
    nc.vector.memset(vaug_sb[:, :, :, 0:1], 1.0)
    nc.vector.memset(vaug_sb[:, :, :, 1:64], 0.0)
    # explicit zero bias for Exp: avoids the shared const-0.0 SBUF tensor,
    # whose region aliases later pool tiles and trips false DMA/ACT races
    zbias = wpool.tile([128, 1], F32, name="zbias")
    nc.vector.memset(zbias[:], 0.0)
    bar_sb = wpool.tile([NC, 16], F32, name="bar_sb")
    nc.vector.memset(bar_sb[:], 0.0)
    nc.sync.dma_start(bar_in[:], bar_sb[:])
    # sacrificial barrier, ready immediately: the first collective after
    # the NEFF init barrier pays an ~11us CC-firmware ramp — let this one
    # absorb it during phase 1 instead of A2A#1.
    nc.gpsimd.collective_compute(
        "AllToAll", mybir.AluOpType.bypass,
        replica_groups=[list(range(NC))],
        ins=[bar_in.opt()], outs=[bar_out.opt()])

    def qk_mm(ps, b, ft, ic, dt):
        nc.tensor.matmul(
            ps[:],
            wqkv_sb[:, dt, 128 * ft:128 * (ft + 1)],
            xt_sb[:, b, dt, 512 * ic:512 * (ic + 1)],
            start=(dt == 0), stop=(dt == 7))

    def vt_mm(ps, b, ic, dt):
        nc.tensor.matmul(
            ps[:],
            wqkv_sb[:, dt, 2 * FS:3 * FS],
            xt_sb[:, b, dt, 512 * ic:512 * (ic + 1)],
            start=(dt == 0), stop=(dt == 7))

    def finish_qk(ps, b, ft, ic):
        nc.vector.tensor_copy(
            qkT_sb[:, ft, b * N + 512 * ic: b * N + 512 * (ic + 1)], ps[:])

    def finish_v(vps_list, b, psum_pool, ptag, pbufs):
        vT_bf = spool.tile([128, N], BF16, tag="vtb", bufs=2, name=f"vtb{b}")
        for ic in range(4):
            nc.vector.tensor_copy(vT_bf[:, 512 * ic:512 * (ic + 1)],
                                  vps_list[ic][:])
        for it in range(16):
            tp = psum_pool.tile([128, 128], BF16, tag=ptag, bufs=pbufs,
                                name=f"t_ps{b}_{it}")
            nc.tensor.transpose(tp[:], vT_bf[:, 128 * it:128 * (it + 1)],
                                ident_sb[:])
            nc.vector.tensor_copy(
                vaug_sb[:, 16 * b + it, :, 64:128],
                tp[:].rearrange("p (h c) -> p h c", h=HPC))

    # ---- warmup while the xT DMA streams in ----
    # preload the ACT exp table (first use costs ~1.3us)
    wsc = spool.tile([128, 1], BF16, tag="wsc", name="wsc")
    nc.scalar.activation(wsc[:], zbias[:], AF.Exp, bias=zbias[:], scale=1.0)

    # ---- Phase 1, batch 0, chunks 0-1 only: dt-outer passes, paced by
    # the xT DMA feed. Chunks 2-3 become pass-0 filler units so the
    # attention pass starts ~20us earlier. ----
    qk_ps = {(ft, ic): pinit.tile([128, 512], F32, tag="init",
                                  bufs=8, name=f"qk0_{ft}_{ic}")
             for ft in range(2) for ic in range(2)}
    v_ps0 = {ic: pinit.tile([128, 512], F32, tag="init", bufs=8,
                            name=f"v0_{ic}") for ic in range(2)}
    for dt in range(8):
        for ft in range(2):
            for ic in range(2):
                qk_mm(qk_ps[ft, ic], 0, ft, ic, dt)
        for ic in range(2):
            vt_mm(v_ps0[ic], 0, ic, dt)
    for ft in range(2):
        for ic in range(2):
            finish_qk(qk_ps[ft, ic], 0, ft, ic)
    vT_bf0 = spool.tile([128, N], BF16, tag="vtb", bufs=2, name="vtb0")
    for ic in range(2):
        nc.vector.tensor_copy(vT_bf0[:, 512 * ic:512 * (ic + 1)],
                              v_ps0[ic][:])
    for it in range(8):
        tp = pinit.tile([128, 128], BF16, tag="init", bufs=8,
                        name=f"t_ps0_{it}")
        nc.tensor.transpose(tp[:], vT_bf0[:, 128 * it:128 * (it + 1)],
                            ident_sb[:])
        nc.vector.tensor_copy(
            vaug_sb[:, it, :, 64:128],
            tp[:].rearrange("p (h c) -> p h c", h=HPC))
    pinit_cm.__exit__(None, None, None)
    ppool_cm = tc.tile_pool(name="psum", bufs=1, space="PSUM")
    ppool = ppool_cm.__enter__()

    # batch-1 x and the out-projection weights are gated behind batch-0's
    # first projection chunk (dummy WAW writes dependent on qkT) so their
    # DMA traffic doesn't compete with the batch-0 load that the phase-1
    # prefix is feed-limited by.
    nc.vector.tensor_copy(xt_sb[:, 1, 0, 0:1], qkT_sb[:, 0, 0:1])
    nc.sync.dma_start(xt_sb[:, 1, :, :], xT[:, 1, :, :])
    nc.vector.tensor_copy(wout_sb[:, 0, 0:1], qkT_sb[:, 0, 0:1])
    nc.sync.dma_start(wout_sb[:], wout[:])

    def p1_units(b, ic_from=0):
        """phase1_seq(b) from chunk ic_from on, decomposed into
        single-matmul emission units so it can be interleaved into a pass2
        as PE filler work. Ordered ic-major (31 units per ic) so a prefix
        makes i-chunk ic of batch b usable."""
        units = []
        state = {}

        def qk_group(ft, ic):
            def alloc():
                state[ft, ic] = ppool.tile([128, 512], F32, tag="mm", bufs=2,
                                           name=f"qk_ps{b}_{ft}_{ic}")
            for dt in range(8):
                def u(ft=ft, ic=ic, dt=dt):
                    if dt == 0:
                        alloc()
                    qk_mm(state[ft, ic], b, ft, ic, dt)
                units.append(u)
            units.append(lambda ft=ft, ic=ic: finish_qk(state[ft, ic], b, ft, ic))

        def v_group(ic):
            def alloc():
                state['v', ic] = ppool.tile([128, 512], F32, tag="mm", bufs=2,
                                            name=f"v_ps{b}_{ic}")
                if ic == ic_from:
                    state['vtb'] = spool.tile([128, N], BF16, tag="vtb",
                                              bufs=2, name=f"vtb{b}")
            for dt in range(8):
                def u(ic=ic, dt=dt):
                    if dt == 0:
                        alloc()
                    vt_mm(state['v', ic], b, ic, dt)
                units.append(u)

            def fin(ic=ic):
                nc.vector.tensor_copy(
                    state['vtb'][:, 512 * ic:512 * (ic + 1)],
                    state['v', ic][:])
            units.append(fin)

        def tr_unit(it):
            def tr(it=it):
                tp = ppool.tile([128, 128], BF16, tag="mm", bufs=2,
                                name=f"t_ps{b}_{it}")
                nc.tensor.transpose(tp[:], state['vtb'][:, 128 * it:128 * (it + 1)],
                                    ident_sb[:])
                nc.vector.tensor_copy(
                    vaug_sb[:, 16 * b + it, :, 64:128],
                    tp[:].rearrange("p (h c) -> p h c", h=HPC))
            units.append(tr)

        for ic in range(ic_from, 4):
            qk_group(0, ic)
            qk_group(1, ic)
            v_group(ic)
            for it in range(4 * ic, 4 * ic + 4):
                tr_unit(it)
        return units

    P1_UNITS_PER_IC = 31

    def proj_units(akey, n_rt, out_b, out_r0):
        """out-projection for 128*n_rt rows from attr_sb[akey]: full w_out
        contraction (8 source K-tiles) into [128,512] psum pairs, as
        filler units. Writes out[out_b, out_r0 + 128*rt ...]."""
        units = []
        state = {}
        for rt in range(n_rt):
            for u in range(8):
                def mm(rt=rt, u=u):
                    if u == 0:
                        for oc in range(2):
                            state[rt, oc] = ppool.tile(
                                [128, 512], F32, tag="mm", bufs=2,
                                name=f"op{akey}_{rt}_{oc}")
                    for oc in range(2):
                        nc.tensor.matmul(
                            state[rt, oc][:],
                            attr_sb[akey][:, u, 128 * rt:128 * (rt + 1)],
                            wout_sb[:, u, 512 * oc:512 * (oc + 1)],
                            start=(u == 0), stop=(u == 7))
                units.append(mm)

            def fin(rt=rt):
                ob = spool.tile([128, D], BF16, tag="ob", bufs=2,
                                name=f"ob{akey}_{rt}")
                for oc in range(2):
                    nc.vector.tensor_copy(ob[:, 512 * oc:512 * (oc + 1)],
                                          state[rt, oc][:])
                nc.sync.dma_start(
                    out[out_b, out_r0 + 128 * rt:out_r0 + 128 * (rt + 1), :],
                    ob[:])
            units.append(fin)
        return units

    last_anf = [None]
    cur_ptp = [None]

    def normalize(b, ic, pvs):
        # denom is pv row 0 (ones row of vaug), per head
        for h in range(HPC):
            if b == 1 and ic == 3:
                # last chunk: nothing follows, and the copy would sit on
                # the exposed A2A#2 trigger chain — normalize from PSUM
                pv = pvs[h, ic]
            else:
                # copy the accumulator out of PSUM first: releases the pv
                # bank immediately so the next chunk's first PV doesn't
                # stall behind the recip/broadcast/mul chain
                praw = spool.tile([128, 512], F32, tag="praw", bufs=4,
                                  name=f"pr{b}_{ic}_{h}")
                nc.vector.tensor_copy(praw[:], pvs[h, ic][:])
                pv = praw
            recip = spool.tile([1, 512], F32, tag="recip", bufs=2,
                               name=f"rc{b}_{ic}_{h}")
            nc.vector.reciprocal_approx_fast(recip[:], pv[0:1, :])
            # partition-broadcast via a DRAM bounce + stride-0-source DMA
            # (SBUF APs reject zero partition stride; DRAM ones don't).
            # NOTHING goes on the GpSimd queue besides the collectives: a
            # gpsimd op between two collectives defers the first one's
            # semaphore increment until the op retires (observed: the
            # A2A#1 gather waiting ~30us past firmware completion behind
            # the chunk-3 broadcast).
            rstage = dpool.tile([1, 512], F32, name=f"rst{b}_{ic}_{h}")
            nc.sync.dma_start(rstage[:], recip[:])
            bc = spool.tile([128, 512], F32, tag="bc", bufs=2,
                            name=f"bc{b}_{ic}_{h}")
            nc.sync.dma_start(bc[64:128, :],
                              rstage[:].to_broadcast((64, 512)))
            bc_sl = bc[64:128, :]
            anf = spool.tile([128, 512], BF16, tag="an", bufs=4,
                             name=f"an{b}_{ic}_{h}")
            nc.vector.tensor_mul(anf[64:128, :], pv[64:128, :], bc_sl)
            last_anf[0] = anf
            # chunk ic covers slots 2ic (cols 0:256) and 2ic+1 (cols
            # 256:512); the slot split is a pure permutation on both sides.
            nc.sync.dma_start(
                a2a_in[b][2 * ic:2 * ic + 2, 64 * h:64 * (h + 1), :]
                .rearrange("s p r -> p s r"),
                anf[64:128, :].rearrange("p (s r) -> p s r", s=2))

    def pass_fused(b, consume=None, at_chunk=None, on_chunk_done=None):
        """causal attention for BOTH heads over all four 512-wide i-chunks
        of batch b, as one software-pipelined stream of j-tiles (the PV of
        tile k is emitted after the S of tile k+1, across chunk
        boundaries). Each j-tile's two heads' S matmuls are concurrent on
        the PE (disjoint row groups). `consume()` emits PE filler work
        once per j-tile; `at_chunk(ic)` runs before each chunk's first
        tile (for prerequisite draining)."""
        plan = [(ic, jt) for ic in range(4) for jt in range(4 * ic + 4)]
        pvs = {}

        def emit_pv(pend):
            pic, pjt, pcp, pptp = pend
            last = (pjt == 4 * pic + 3)
            for h in range(HPC):
                nc.tensor.matmul(pvs[h, pic][:, pcp:512],
                                 vaug_sb[:, 16 * b + pjt, h, :],
                                 pptp[:, h, pcp:512],
                                 start=(pjt == 0), stop=last)
            if last:
                normalize(b, pic, pvs)
                if on_chunk_done is not None:
                    on_chunk_done(pic)

        pend = []
        for ic, jt in plan:
            if jt == 0:
                if at_chunk is not None:
                    at_chunk(ic)
                for h in range(HPC):
                    pvs[h, ic] = ppool.tile([128, 512], F32, tag="pv",
                                            bufs=2, name=f"pv{b}_{ic}_{h}")
            q0 = jt - 4 * ic
            cp = 128 * q0 if q0 > 0 else 0
            sp = ppool.tile([128, 2, 512], F32, tag="s", bufs=2,
                            name=f"s{b}_{ic}_{jt}")
            ptp = ptpool.tile([128, 2, 512], BF16, tag="pt", bufs=16,
                              name=f"pt{b}_{ic}_{jt}")
            for h in range(HPC):
                nc.tensor.matmul(
                    sp[:, h, cp:512],
                    qkT_sb[64 * h:64 * (h + 1), 1,
                           b * N + 128 * jt: b * N + 128 * (jt + 1)],
                    qkT_sb[64 * h:64 * (h + 1), 0,
                           b * N + 512 * ic + cp: b * N + 512 * (ic + 1)],
                    start=True, stop=True)
            nc.scalar.activation(ptp[:, :, cp:512], sp[:, :, cp:512],
                                 AF.Exp, bias=zbias[:], scale=SCALE)
            if q0 >= 0:
                # diagonal tile: only the 128-col diagonal block needs the
                # causal mask (columns right of it are fully valid)
                nc.vector.tensor_mul(ptp[:, :, cp:cp + 128],
                                     ptp[:, :, cp:cp + 128], maskblk_sb[:])
            cur_ptp[0] = ptp
            if consume is not None:
                consume()
            # depth-2 software pipeline: the PV of tile k is emitted after
            # the S of tile k+2, so at chunk boundaries the next chunk's
            # first S/exp outrank the previous chunk's PV backlog in the
            # scheduler's program-order priority.
            if len(pend) >= 2:
                emit_pv(pend.pop(0))
            pend.append((ic, jt, cp, ptp))
        while pend:
            emit_pv(pend.pop(0))

    def do_a2a(key):
        nc.gpsimd.collective_compute(
            "AllToAll", mybir.AluOpType.bypass,
            replica_groups=[list(range(NC))],
            ins=[a2a_in[key].opt()], outs=[a2a_out[key].opt()])
        # gather [8,128,R] -> attr_sb[key] [128, 8, R]. Dispatched from
        # the GpSimd queue: it sits right after its own collective there
        # and fires the moment it completes (on the Sync queue these ended
        # up serialized behind LATER collectives' cumulative thresholds).
        eng = nc.scalar if key == 0 else nc.sync
        for half in range(2):
            eng.dma_start(
                attr_sb[key][:, 4 * half:4 * half + 4, :],
                a2a_out[key][4 * half:4 * half + 4]
                .rearrange("u p r -> p u r"))

    # ---- Phase 2, batch 0 ----
    # batch-0 chunks 2-3 QKV prep and then batch-1 QKV/V prep ride along
    # as PE filler; leftovers drain inside the batch-1 pass (force-drained
    # just in time per chunk).
    units0 = p1_units(0, ic_from=2)
    done0 = [0]
    units1 = p1_units(1)
    done1 = [0]

    def consume_p1(k):
        while k > 0:
            if done0[0] < len(units0):
                units0[done0[0]]()
                done0[0] += 1
            elif done1[0] < len(units1):
                units1[done1[0]]()
                done1[0] += 1
            else:
                break
            k -= 1

    def at_chunk_b0(ic):
        # chunk ic's S matmuls need q/k chunks <= ic and vaug tiles
        # <= 4*ic+3 of batch 0; units0 covers chunks 2-3 ic-major.
        need = P1_UNITS_PER_IC * max(0, ic - 1)
        consume_b0 = max(0, need - done0[0])
        while consume_b0 > 0 and done0[0] < len(units0):
            units0[done0[0]]()
            done0[0] += 1
            consume_b0 -= 1

    pass_fused(0, consume=lambda: consume_p1(3), at_chunk=at_chunk_b0)
    do_a2a(0)

    # ---- Phase 2, batch 1 ----
    # p1 leftovers fill the early tiles. The batch-0 out-projection is NOT
    # consumed in-pass (A_GATE=999): collective completion only becomes
    # engine-visible ~30us after the firmware finishes moving data, so its
    # A2A#1-gated units never actually engage mid-pass — they all drain at
    # the tail, where they land anyway.
    unitsA = proj_units(0, 2, 0, 0)
    doneA = [0]
    jt_ctr = [0]
    A_GATE = 24

    # hold back the last row-tile group of the batch-0 out-projection: it
    # runs DURING the exposed A2A#2 window, keeping the PE p-state warm so
    # the batch-1 out-projection starts at full clock instead of ramping
    A_RESERVE = 9

    def consume_b1():
        jt_ctr[0] += 1
        if done1[0] < len(units1):
            consume_p1(2)
        elif jt_ctr[0] > A_GATE:
            for _ in range(2):
                if doneA[0] < len(unitsA) - A_RESERVE:
                    unitsA[doneA[0]]()
                    doneA[0] += 1

    def at_chunk_b1(ic):
        consume_p1(max(0, P1_UNITS_PER_IC * (ic + 1) - done1[0]))

    # no pre-tail barrier: it CC-serializes ahead of A2A#2, so when cores
    # ARE skewed its own (inflated) duration lands on the critical path —
    # letting A2A#2 absorb the skew directly costs no extra serialization
    pass_fused(1, consume=consume_b1, at_chunk=at_chunk_b1)

    # ---- Phase 3 tail: leftover batch-0 out-projection units, then the
    # A2A#2 (with the reserved units filling its window) + batch-1
    # out-projection + writeback ----
    while doneA[0] < len(unitsA) - A_RESERVE:
        unitsA[doneA[0]]()
        doneA[0] += 1
    do_a2a(1)
    while doneA[0] < len(unitsA):
        unitsA[doneA[0]]()
        doneA[0] += 1
    for u in proj_units(1, 2, 1, 0):
        u()

    for p in reversed(ctx_pools):
        p.__exit__(None, None, None)


def _host_inputs(x, w_qkv, w_out):
    x = np.asarray(x, dtype=np.float32)
    w_qkv = np.asarray(w_qkv, dtype=np.float32)
    w_out = np.asarray(w_out, dtype=np.float32)

    # xT[p, b, dt, i] = x[b, i, 128*dt + p]
    xTt = np.ascontiguousarray(
        x.transpose(2, 0, 1).reshape(8, 128, NB, N).transpose(1, 2, 0, 3)
    ).astype(ml_dtypes.bfloat16)

    wq, wk, wv = w_qkv[:, 0:D], w_qkv[:, D:2 * D], w_qkv[:, 2 * D:3 * D]

    # wout3[p, u, :] = w_out[128*u + p, :]
    wout3 = np.ascontiguousarray(
        w_out.reshape(8, 128, D).transpose(1, 0, 2)).astype(ml_dtypes.bfloat16)

    # diagonal-block causal mask, same for every diagonal j-tile:
    # keep iff (query col within block) >= (key partition)
    k_i = np.arange(128)[:, None]
    c_i = np.arange(128)[None, :]
    mblk = (c_i >= k_i)
    maskblk = np.ascontiguousarray(
        np.stack([mblk, mblk], axis=1)).astype(ml_dtypes.bfloat16)
    identity = np.eye(128, dtype=ml_dtypes.bfloat16)

    in_maps = []
    for c in range(NC):
        sl = slice(FS * c, FS * (c + 1))
        wq_c = np.concatenate([wq[:, sl], wk[:, sl], wv[:, sl]], axis=1)
        wq_c = np.ascontiguousarray(
            wq_c.astype(ml_dtypes.bfloat16).reshape(8, 128, 3 * FS)
            .transpose(1, 0, 2))
        in_maps.append({
            "xT": xTt,
            "wqkv": wq_c,
            "wout": wout3,
            "maskblk": maskblk,
            "ident": identity,
        })
    return in_maps


def run_hw(inputs, trace=False, **kw):
    """Run on 8 NeuronCores. Returns (full_output, BassKernelResults)."""
    global _CACHED_NC
    if _CACHED_NC is None:
        _CACHED_NC = build_graph()
    in_maps = _host_inputs(inputs["x"], inputs["w_qkv"], inputs["w_out"])
    res = run_bass_kernel_spmd(_CACHED_NC, in_maps,
                               core_ids=list(range(NC)), trace=trace, **kw)
    # core c's out is [NB, 256, D] = rows [256c, 256c+256) of each batch
    y = np.concatenate([np.asarray(res.results[c]["out"]) for c in range(NC)],
                       axis=1).astype(np.float32)
    return y, res


def kernel(**inputs):
    y, _ = run_hw(inputs, trace=bool(os.environ.get("BASS_TRACE")))
    return y
